# revision 1
# baseline (speedup 1.0000x reference)
"""HGCN (hyperbolic GCN) 2-layer forward for Trainium2, 8 NeuronCores.

Strategy (graph-parallel, dense-spmm):
  - Nodes padded 10000 -> 10240 and sharded 1280/core (8 cores).
  - segment_sum over the edge list is recast as a dense matmul
    agg = A @ xt with A[dst, src] = sum of edge weights; A is built on the
    host from edge_index/edge_weight and each core streams its
    [10240 x 1280] column slice of A^T (k-major tiles) from HBM.
  - Per layer: HypLinear + logmap0 run on the core's own 1280 nodes,
    the [1280, 256] tangent features are AllGathered (DRAM bounce),
    the spmm accumulates 10 PSUM tiles over 80 k-tiles, and HypAct
    (expmap0/proj/relu-logmap/expmap0/proj) finishes in place.
  - All per-node scalar chains (norms, artanh, tanh, mobius coeffs) are
    batched as [128, 10] column arrays to amortize instruction overhead.

kernel(**inputs) takes the FULL unsharded inputs and returns [2, N, D].
"""

import sys

import numpy as np

for _p in ("/opt/trn_rl_repo",):
    if _p not in sys.path:
        sys.path.append(_p)

import concourse.bass as bass  # noqa: E402
import concourse.tile as tile  # noqa: E402
from concourse import bacc, mybir  # noqa: E402
from concourse.bass_utils import run_bass_kernel_spmd  # noqa: E402
from concourse.masks import make_identity  # noqa: E402

AF = mybir.ActivationFunctionType
ALU = mybir.AluOpType
F32 = mybir.dt.float32

NCORES = 8
N = 10000
D = 256
NP = 10240
PC = NP // NCORES      # 1280 nodes per core
NT = PC // 128         # 10 node tiles per core
KT = NP // 128         # 80 contraction tiles
MAXN = 1.0 - 4e-3      # PROJ_EPS clip for c=1
MINN = 1e-15
MM_DT = "bfloat16"     # dtype of the spmm operands ("float32" | "bfloat16")
DEBUG = False          # add intermediate dumps for layer 0


def _mm_np_dtype():
    if MM_DT == "bfloat16":
        import ml_dtypes

        return np.dtype(ml_dtypes.bfloat16)
    return np.dtype(np.float32)


def _mm_bir_dtype():
    return mybir.dt.bfloat16 if MM_DT == "bfloat16" else F32


def build_nc(y2s):
    """Build the per-core Bass program. y2s = (||hyp_b1||^2, ||hyp_b2||^2)."""
    mmdt = _mm_bir_dtype()
    nc = bacc.Bacc("TRN2", target_bir_lowering=False, debug=False,
                   num_devices=NCORES)

    xc = nc.dram_tensor("xc", [NT, 128, D], F32, kind="ExternalInput")
    a_d = nc.dram_tensor("a", [KT, 128, PC], mmdt, kind="ExternalInput")
    w1t = nc.dram_tensor("w1t", [2, 128, D], F32, kind="ExternalInput")
    w2t = nc.dram_tensor("w2t", [2, 128, D], F32, kind="ExternalInput")
    hb1 = nc.dram_tensor("hb1", [128, D], F32, kind="ExternalInput")
    hb2 = nc.dram_tensor("hb2", [128, D], F32, kind="ExternalInput")
    e1_d = nc.dram_tensor("e1", [NT, 128, D], F32, kind="ExternalOutput")
    e2_d = nc.dram_tensor("e2", [NT, 128, D], F32, kind="ExternalOutput")
    dbg = {}
    if DEBUG:
        for nm, shp in [("dbg_h", [NT, 128, D]), ("dbg_mx", [NT, 128, D]),
                        ("dbg_xt", [NT, 128, D]), ("dbg_agg", [NT, 128, D]),
                        ("dbg_xtf", [KT, 128, D])]:
            dbg[nm] = nc.dram_tensor(nm, shp, F32, kind="ExternalOutput")

    with tile.TileContext(nc) as tc:
        with (
            tc.tile_pool(name="const", bufs=1) as const,
            tc.tile_pool(name="persist", bufs=1) as persist,
            tc.tile_pool(name="sqp", bufs=3) as sqp,
            tc.tile_pool(name="htp", bufs=4) as htp,
            tc.tile_pool(name="atp", bufs=16) as atp,
            tc.tile_pool(name="pst", bufs=2, space="PSUM") as pst,
            tc.tile_pool(name="psmx", bufs=1, space="PSUM") as psmx,
            tc.tile_pool(name="psagg", bufs=1, space="PSUM") as psagg,
            tc.tile_pool(name="dram", bufs=1, space="DRAM") as dram,
        ):
            ident = const.tile([128, 128], F32, name="ident")
            make_identity(nc, ident)

            w_sb = []
            for li, wd in enumerate((w1t, w2t)):
                w = const.tile([128, 2, D], F32, name=f"w{li}")
                nc.sync.dma_start(w[:], wd.ap().rearrange("k p n -> p k n"))
                w_sb.append(w)
            hb_sb = []
            for li, hd in enumerate((hb1, hb2)):
                h = const.tile([128, D], F32, name=f"hb{li}")
                nc.sync.dma_start(h[:], hd.ap())
                hb_sb.append(h)

            def sc(name):
                return persist.tile([128, NT], F32, name=name)

            def square_accum(src_ap, accum_ap, name):
                s = sqp.tile([128, D], F32, name="sqt", tag="sqt")
                nc.scalar.activation(s[:], src_ap, AF.Square, accum_out=accum_ap)

            def clamp_recip(dst, src, name):
                c = sc(name + "_c")
                nc.vector.tensor_scalar_max(c[:], src[:], MINN)
                nc.vector.reciprocal(dst[:], c[:])

            def artanh_ln(dst, x, name):
                """dst = ln((1+x)/(1-x)); caller owns the 0.5 factor."""
                ap1 = sc(name + "_ap")
                am1 = sc(name + "_am")
                ram = sc(name + "_ram")
                q = sc(name + "_q")
                nc.scalar.activation(ap1[:], x[:], AF.Identity, bias=1.0)
                nc.scalar.activation(am1[:], x[:], AF.Identity, bias=1.0, scale=-1.0)
                nc.vector.reciprocal(ram[:], am1[:])
                nc.vector.tensor_tensor(q[:], ap1[:], ram[:], ALU.mult)
                nc.scalar.activation(dst[:], q[:], AF.Ln)

            # ---------------- encode: h = proj(expmap0(x)) ----------------
            x_sb = persist.tile([128, NT, D], F32, name="x_sb", tag="bigA")
            nc.sync.dma_start(x_sb[:], xc.ap().rearrange("t p d -> p t d"))
            h_all = persist.tile([128, NT, D], F32, name="h_all", tag="bigB")
            xn2 = sc("xn2")
            for t in range(NT):
                square_accum(x_sb[:, t, :], xn2[:, t : t + 1], f"enc{t}")
            un = sc("un")
            nc.scalar.activation(un[:], xn2[:], AF.Sqrt)
            run_ = sc("run")
            clamp_recip(run_, un, "enc_r")
            thx = sc("thx")
            nc.scalar.activation(thx[:], un[:], AF.Tanh)
            mn0 = sc("mn0")
            nc.vector.tensor_scalar_min(mn0[:], thx[:], MAXN)
            s0 = sc("s0")
            nc.vector.tensor_tensor(s0[:], mn0[:], run_[:], ALU.mult)
            for t in range(NT):
                nc.vector.tensor_scalar_mul(h_all[:, t, :], x_sb[:, t, :],
                                            s0[:, t : t + 1])

            def layer(li, h_in, hnorm, e_out_d):
                """One HGCN layer; h_in [128,NT,D] on-ball, hnorm [128,NT] its
                row norms. Returns (e_all, out_norms)."""
                L = f"l{li}_"
                w = w_sb[li]
                hb = hb_sb[li]
                y2 = float(y2s[li])

                # ---- HypLinear matmuls + |mx|^2 ----
                mx_all = persist.tile([128, NT, D], F32, name=L + "mx", tag="bigA")
                mn2 = sc(L + "mn2")
                for t in range(NT):
                    hT = htp.tile([128, 2, 128], F32, name="hT", tag="hT")
                    for kc in range(2):
                        psT = pst.tile([128, 128], F32, name="psT", tag="psT")
                        nc.tensor.transpose(
                            psT[:], h_in[:, t, kc * 128 : (kc + 1) * 128], ident[:])
                        nc.vector.tensor_copy(hT[:, kc, :], psT[:])
                    pmx = psmx.tile([128, D], F32, name="pmx", tag="pmx")
                    nc.tensor.matmul(pmx[:], hT[:, 0, :], w[:, 0, :],
                                     start=True, stop=False)
                    nc.tensor.matmul(pmx[:], hT[:, 1, :], w[:, 1, :],
                                     start=False, stop=True)
                    square_accum(pmx[:], mn2[:, t : t + 1], L + f"mx{t}")
                    nc.vector.tensor_copy(mx_all[:, t, :], pmx[:])

                # ---- SB1: mobius_matvec scalars ----
                mxn = sc(L + "mxn")
                nc.scalar.activation(mxn[:], mn2[:], AF.Sqrt)
                nc.vector.tensor_scalar_max(mxn[:], mxn[:], MINN)
                rxn = sc(L + "rxn")
                clamp_recip(rxn, hnorm, L + "rxn")
                rmxn = sc(L + "rmxn")
                nc.vector.reciprocal(rmxn[:], mxn[:])
                atx = sc(L + "atx")
                artanh_ln(atx, hnorm, L + "atx")
                targ = sc(L + "targ")
                nc.vector.tensor_tensor(targ[:], mxn[:], rxn[:], ALU.mult)
                nc.vector.tensor_tensor(targ[:], targ[:], atx[:], ALU.mult)
                th = sc(L + "th")
                nc.scalar.activation(th[:], targ[:], AF.Tanh, scale=0.5)
                sres = sc(L + "sres")
                nc.vector.tensor_tensor(sres[:], th[:], rmxn[:], ALU.mult)
                # proj of res: norm is th (analytically); f1 = min(MAXN/th, 1)
                rth = sc(L + "rth")
                clamp_recip(rth, th, L + "rth")
                f1 = sc(L + "f1")
                nc.vector.tensor_scalar(f1[:], rth[:], MAXN, 1.0, ALU.mult, ALU.min)
                nres = sc(L + "nres")
                nc.vector.tensor_scalar_min(nres[:], th[:], MAXN)
                x2 = sc(L + "x2")
                nc.vector.tensor_tensor(x2[:], nres[:], nres[:], ALU.mult)

                # ---- per tile: xy accumulation (on unscaled mx) ----
                ryp = sc(L + "ryp")
                for t in range(NT):
                    prod = sqp.tile([128, D], F32, name="prodt", tag="prodt")
                    nc.vector.tensor_tensor(prod[:], mx_all[:, t, :], hb[:],
                                            ALU.mult)
                    nc.vector.reduce_sum(ryp[:, t : t + 1], prod[:],
                                         axis=mybir.AxisListType.X)

                # ---- SB2: mobius_add coefficients ----
                xy = sc(L + "xy")
                nc.vector.tensor_tensor(xy[:], ryp[:], sres[:], ALU.mult)
                nc.vector.tensor_tensor(xy[:], xy[:], f1[:], ALU.mult)
                apre = sc(L + "apre")
                nc.vector.tensor_scalar(apre[:], xy[:], 2.0, 1.0 + y2,
                                        ALU.mult, ALU.add)
                alpha = sc(L + "alpha")
                nc.vector.tensor_tensor(alpha[:], apre[:], f1[:], ALU.mult)
                beta = sc(L + "beta")
                nc.scalar.activation(beta[:], x2[:], AF.Identity,
                                     bias=1.0, scale=-1.0)
                den = sc(L + "den")
                nc.vector.tensor_scalar(den[:], x2[:], y2, 1.0, ALU.mult, ALU.add)
                xy2 = sc(L + "xy2")
                nc.vector.tensor_scalar_mul(xy2[:], xy[:], 2.0)
                nc.vector.tensor_tensor(den[:], den[:], xy2[:], ALU.add)
                dinv = sc(L + "dinv")
                clamp_recip(dinv, den, L + "dinv")
                asc = sc(L + "asc")
                nc.vector.tensor_tensor(asc[:], alpha[:], dinv[:], ALU.mult)
                nc.vector.tensor_tensor(asc[:], asc[:], sres[:], ALU.mult)
                bsc = sc(L + "bsc")
                nc.vector.tensor_tensor(bsc[:], beta[:], dinv[:], ALU.mult)

                # ---- per tile: h2 = asc*mx + bsc*hb ; |h2|^2 ----
                h2_all = persist.tile([128, NT, D], F32, name=L + "h2", tag="bigB")
                hn2 = sc(L + "hn2")
                for t in range(NT):
                    t1 = sqp.tile([128, D], F32, name="t1t", tag="t1t")
                    nc.vector.tensor_scalar_mul(t1[:], mx_all[:, t, :],
                                                asc[:, t : t + 1])
                    t2 = sqp.tile([128, D], F32, name="t2t", tag="t2t")
                    nc.scalar.activation(t2[:], hb[:], AF.Copy,
                                         scale=bsc[:, t : t + 1])
                    nc.vector.tensor_tensor(h2_all[:, t, :], t1[:], t2[:], ALU.add)
                    square_accum(h2_all[:, t, :], hn2[:, t : t + 1], L + f"h2{t}")

                # ---- SB3: proj + logmap0 scale ----
                hn = sc(L + "hn")
                nc.scalar.activation(hn[:], hn2[:], AF.Sqrt)
                rhn = sc(L + "rhn")
                clamp_recip(rhn, hn, L + "rhn")
                f2 = sc(L + "f2")
                nc.vector.tensor_scalar(f2[:], rhn[:], MAXN, 1.0, ALU.mult, ALU.min)
                m = sc(L + "m")
                nc.vector.tensor_scalar_min(m[:], hn[:], MAXN)
                rm = sc(L + "rm")
                clamp_recip(rm, m, L + "rm")
                atm = sc(L + "atm")
                artanh_ln(atm, m, L + "atm")
                g = sc(L + "g")
                nc.vector.tensor_tensor(g[:], atm[:], rm[:], ALU.mult)
                nc.vector.tensor_tensor(g[:], g[:], f2[:], ALU.mult)
                nc.vector.tensor_scalar_mul(g[:], g[:], 0.5)

                # ---- per tile: xt = g * h2 (tangent features) ----
                mmdt_ = _mm_bir_dtype()
                xt_all = persist.tile([128, NT, D], mmdt_, name=L + "xt", tag="bigC")
                for t in range(NT):
                    nc.vector.tensor_scalar_mul(xt_all[:, t, :], h2_all[:, t, :],
                                                g[:, t : t + 1])

                if DEBUG and li == 0:
                    nc.sync.dma_start(dbg["dbg_mx"].ap().rearrange("t p d -> p t d"),
                                      mx_all[:])
                    nc.sync.dma_start(dbg["dbg_xt"].ap().rearrange("t p d -> p t d"),
                                      xt_all[:])
                # ---- AllGather tangent features, chunked and overlapped ----
                # Split the gather into NCH chunks of TPC local tiles so the
                # spmm over chunk j runs while chunk j+1 is still gathering
                # (collectives run on TOPSP/SDMA, free of the 5 engines).
                NCH = 5
                TPC = NT // NCH
                xt_full = persist.tile([128, KT, D], mmdt_, name="xt_full",
                                       tag="xt_full")
                xtf_view = xt_full[:].rearrange("p (c t) d -> p c t d", t=NT)
                for j in range(NCH):
                    agin = dram.tile([TPC, 128, D], mmdt_, name=f"{L}agin{j}",
                                     tag=f"agin{j}")
                    agout = dram.tile([NCORES * TPC, 128, D], mmdt_,
                                      name=f"{L}agout{j}", tag=f"agout{j}",
                                      addr_space="Shared")
                    nc.sync.dma_start(agin[:].rearrange("t p d -> p t d"),
                                      xt_all[:, j * TPC : (j + 1) * TPC, :])
                    nc.gpsimd.collective_compute(
                        "AllGather", ALU.bypass,
                        replica_groups=[list(range(NCORES))],
                        ins=[agin[:].opt()], outs=[agout[:].opt()])
                    agout_v = agout[:].rearrange("(c t) p d -> p c t d", t=TPC)
                    for i in range(TPC):
                        nc.sync.dma_start(
                            xtf_view[:, :, j * TPC + i, :],
                            agout_v[:, :, i, :])
                if DEBUG and li == 0:
                    nc.sync.dma_start(dbg["dbg_xtf"].ap().rearrange("t p d -> p t d"),
                                      xt_full[:])

                # ---- spmm: agg[dst, f] = sum_src AT[src, dst] xt[src, f] ----
                # k-tiles grouped by AG chunk: chunk j supplies global k-tiles
                # {c*NT + j*TPC + i}. PSUM 'start' clears the whole 2KB bank;
                # tiles t, t+1 share a bank, so only the very first matmul of
                # each even tile issues start=True.
                pagg = psagg.tile([128, NT, D], F32, name="pagg", tag="pagg")
                k_iter = 0
                for j in range(NCH):
                    for c in range(NCORES):
                        for i in range(TPC):
                            kt = c * NT + j * TPC + i
                            at_k = atp.tile([128, PC], mmdt_, name="at_k",
                                            tag="at_k")
                            nc.sync.dma_start(at_k[:], a_d.ap()[kt])
                            for t in range(NT):
                                nc.tensor.matmul(
                                    pagg[:, t, :],
                                    at_k[:, t * 128 : (t + 1) * 128],
                                    xt_full[:, kt, :],
                                    start=(k_iter == 0 and t % 2 == 0),
                                    stop=(k_iter == KT - 1),
                                    skip_group_check=True)
                            k_iter += 1

                # ---- HypAct ----
                if DEBUG and li == 0:
                    agg_sb = persist.tile([128, NT, D], F32, name="agg_sb")
                    for t in range(NT):
                        nc.vector.tensor_copy(agg_sb[:, t, :], pagg[:, t, :])
                    nc.sync.dma_start(dbg["dbg_agg"].ap().rearrange("t p d -> p t d"),
                                      agg_sb[:])
                r2 = sc(L + "r2")
                for t in range(NT):
                    square_accum(pagg[:, t, :], r2[:, t : t + 1], L + f"agg{t}")
                rn = sc(L + "rn")
                nc.scalar.activation(rn[:], r2[:], AF.Sqrt)
                rrn = sc(L + "rrn")
                clamp_recip(rrn, rn, L + "rrn")
                th2 = sc(L + "th2")
                nc.scalar.activation(th2[:], rn[:], AF.Tanh)
                m1 = sc(L + "m1")
                nc.vector.tensor_scalar_min(m1[:], th2[:], MAXN)
                rm1 = sc(L + "rm1")
                clamp_recip(rm1, m1, L + "rm1")
                s1 = sc(L + "s1")
                nc.vector.tensor_tensor(s1[:], m1[:], rrn[:], ALU.mult)
                atq = sc(L + "atq")
                artanh_ln(atq, m1, L + "atq")
                qs = sc(L + "qs")
                nc.vector.tensor_tensor(qs[:], s1[:], atq[:], ALU.mult)
                nc.vector.tensor_tensor(qs[:], qs[:], rm1[:], ALU.mult)
                nc.vector.tensor_scalar_mul(qs[:], qs[:], 0.5)

                xt2_all = persist.tile([128, NT, D], F32, name=L + "xt2", tag="bigD")
                n2b = sc(L + "n2b")
                for t in range(NT):
                    # relu(qs*agg) on DVE (PSUM src), freeing the ACT engine
                    nc.vector.tensor_scalar(xt2_all[:, t, :], pagg[:, t, :],
                                            qs[:, t : t + 1], 0.0,
                                            ALU.mult, ALU.max)
                    square_accum(xt2_all[:, t, :], n2b[:, t : t + 1], L + f"xb{t}")

                un2 = sc(L + "un2")
                nc.scalar.activation(un2[:], n2b[:], AF.Sqrt)
                run2 = sc(L + "run2")
                clamp_recip(run2, un2, L + "run2")
                th3 = sc(L + "th3")
                nc.scalar.activation(th3[:], un2[:], AF.Tanh)
                mm2 = sc(L + "mm2")
                nc.vector.tensor_scalar_min(mm2[:], th3[:], MAXN)
                ss = sc(L + "ss")
                nc.vector.tensor_tensor(ss[:], mm2[:], run2[:], ALU.mult)

                e_all = persist.tile([128, NT, D], F32, name=L + "e", tag="bigE")
                for t in range(NT):
                    nc.vector.tensor_scalar_mul(e_all[:, t, :], xt2_all[:, t, :],
                                                ss[:, t : t + 1])
                nc.sync.dma_start(e_out_d.ap().rearrange("t p d -> p t d"),
                                  e_all[:])
                return e_all, mm2

            if DEBUG:
                nc.sync.dma_start(dbg["dbg_h"].ap().rearrange("t p d -> p t d"),
                                  h_all[:])
            e1_all, n1 = layer(0, h_all, mn0, e1_d)
            layer(1, e1_all, n1, e2_d)

    nc.compile()
    return nc


def _hyp_bias(b):
    """proj(expmap0(b, c=1), c=1) in float32, mirroring the reference."""
    b = b.astype(np.float32)
    un = np.maximum(np.sqrt((b * b).sum()), np.float32(MINN)).astype(np.float32)
    h = (np.tanh(un) * b / un).astype(np.float32)
    n = np.maximum(np.sqrt((h * h).sum()), np.float32(MINN)).astype(np.float32)
    if n > np.float32(MAXN):
        h = (h / n * np.float32(MAXN)).astype(np.float32)
    return h


def prepare_inputs(x, W1, b1, W2, b2, edge_index, edge_weight):
    mmnp = _mm_np_dtype()
    x = np.asarray(x, np.float32)
    W1 = np.asarray(W1, np.float32)
    W2 = np.asarray(W2, np.float32)
    b1 = np.asarray(b1, np.float32)
    b2 = np.asarray(b2, np.float32)
    ew = np.asarray(edge_weight, np.float32)
    src = np.asarray(edge_index[0], np.int64)
    dst = np.asarray(edge_index[1], np.int64)

    AT = np.zeros((NP, NP), np.float32)
    np.add.at(AT, (src, dst), ew)

    xfull = np.zeros((NP, D), np.float32)
    xfull[:N] = x

    hb1 = _hyp_bias(b1)
    hb2 = _hyp_bias(b2)
    y2s = (float((hb1.astype(np.float64) ** 2).sum()),
           float((hb2.astype(np.float64) ** 2).sum()))

    w1t = np.ascontiguousarray(W1.T).reshape(2, 128, D)
    w2t = np.ascontiguousarray(W2.T).reshape(2, 128, D)
    hb1_b = np.tile(hb1[None, :], (128, 1)).astype(np.float32)
    hb2_b = np.tile(hb2[None, :], (128, 1)).astype(np.float32)

    in_maps = []
    for c in range(NCORES):
        ac = np.ascontiguousarray(
            AT[:, c * PC : (c + 1) * PC]).reshape(KT, 128, PC).astype(mmnp)
        xcr = xfull[c * PC : (c + 1) * PC].reshape(NT, 128, D)
        in_maps.append({
            "xc": np.ascontiguousarray(xcr),
            "a": ac,
            "w1t": w1t, "w2t": w2t,
            "hb1": hb1_b, "hb2": hb2_b,
        })
    return in_maps, y2s


def assemble(results):
    e1 = np.concatenate([r["e1"].reshape(PC, D) for r in results], 0)[:N]
    e2 = np.concatenate([r["e2"].reshape(PC, D) for r in results], 0)[:N]
    return np.stack([e1, e2], 0).astype(np.float32)


def run(inputs, trace=False):
    in_maps, y2s = prepare_inputs(**inputs)
    nc = build_nc(y2s)
    res = run_bass_kernel_spmd(nc, in_maps, core_ids=list(range(NCORES)),
                               trace=trace)
    return assemble(res.results), res


def kernel(**inputs):
    out, _ = run(inputs, trace=False)
    return out



# revision 25
# speedup vs baseline: 1.0822x; 1.0822x over previous
"""HGCN (hyperbolic GCN) 2-layer forward for Trainium2, 8 NeuronCores.

Strategy (graph-parallel, dense-spmm):
  - Nodes padded 10000 -> 10240 and sharded 1280/core (8 cores).
  - segment_sum over the edge list is recast as a dense matmul
    agg = A @ xt with A[dst, src] = sum of edge weights; A is built on the
    host and each core owns its [10240 x 1280] column slice in fp16.
  - Half of A (the k-tiles gated by AllGather chunk 0) stays resident in
    SBUF across both layers; the other half streams from HBM on the sync
    HWDGE ring while AG bounce traffic uses the scalar HWDGE ring.
  - Per layer: HypLinear + tangent-map run on the core's own 1280 nodes,
    the [1280, 256] fp16 tangent features are AllGathered in 2 chunks
    (big chunks amortize the ~5-15us collective floor), the spmm
    accumulates 10 PSUM tiles over 80 k-tiles back-to-back so the PE
    stays at max p-state, and HypAct finishes in place.
  - Per-node scalars are batched as [128, 10] column arrays; norms use
    fused DVE tensor_tensor_reduce; |h2|^2 is computed analytically from
    |mx|^2, mx.hb and |hb|^2 so h2 is never materialized.

kernel(**inputs) takes the FULL unsharded inputs and returns [2, N, D].
"""

import sys

import numpy as np

for _p in ("/opt/trn_rl_repo",):
    if _p not in sys.path:
        sys.path.append(_p)

import concourse.bass as bass  # noqa: E402
import concourse.tile as tile  # noqa: E402
from concourse import bacc, mybir  # noqa: E402
from concourse.bass_utils import run_bass_kernel_spmd  # noqa: E402
from concourse.masks import make_identity  # noqa: E402

AF = mybir.ActivationFunctionType
ALU = mybir.AluOpType
F32 = mybir.dt.float32
F16 = mybir.dt.float16

NCORES = 8
N = 10000
D = 256
NP = 10240
PC = NP // NCORES      # 1280 nodes per core
NT = PC // 128         # 10 node tiles per core
KT = NP // 128         # 80 contraction tiles
NCH = 2                # AllGather chunks per layer
TPC = NT // NCH        # local tiles per AG chunk (5)
RESN = 0               # resident k-tiles (SBUF), rest stream per layer
MAXN = 1.0 - 4e-3      # PROJ_EPS clip for c=1
MINN = 1e-15


def build_nc(y2s):
    """Build the per-core Bass program. y2s = (||hyp_b1||^2, ||hyp_b2||^2)."""
    nc = bacc.Bacc("TRN2", target_bir_lowering=False, debug=False,
                   num_devices=NCORES)

    xc = nc.dram_tensor("xc", [NT, 128, D], F32, kind="ExternalInput")
    ares_d = nc.dram_tensor("ares", [max(RESN, 1), 128, PC], F16,
                            kind="ExternalInput")
    astr_d = nc.dram_tensor("astr", [KT - RESN, 128, PC], F16,
                            kind="ExternalInput")
    w1t = nc.dram_tensor("w1t", [2, 128, D], F32, kind="ExternalInput")
    w2t = nc.dram_tensor("w2t", [2, 128, D], F32, kind="ExternalInput")
    hb1 = nc.dram_tensor("hb1", [128, D], F32, kind="ExternalInput")
    hb2 = nc.dram_tensor("hb2", [128, D], F32, kind="ExternalInput")
    u1 = nc.dram_tensor("u1", [128, D], F32, kind="ExternalInput")
    u2 = nc.dram_tensor("u2", [128, D], F32, kind="ExternalInput")
    e1_d = nc.dram_tensor("e1", [NT, 128, D], F32, kind="ExternalOutput")
    e2_d = nc.dram_tensor("e2", [NT, 128, D], F32, kind="ExternalOutput")

    with tile.TileContext(nc) as tc:
        with (
            tc.tile_pool(name="const", bufs=1) as const,
            tc.tile_pool(name="persist", bufs=1) as persist,
            tc.tile_pool(name="sqp", bufs=4) as sqp,
            tc.tile_pool(name="htp", bufs=2) as htp,
            tc.tile_pool(name="atp", bufs=12) as atp,
            tc.tile_pool(name="pst", bufs=2, space="PSUM") as pst,
            tc.tile_pool(name="psmx", bufs=1, space="PSUM") as psmx,
            tc.tile_pool(name="psagg", bufs=1, space="PSUM") as psagg,
            tc.tile_pool(name="dram", bufs=1, space="DRAM") as dram,
        ):
            # ---- const loads (sync HWDGE ring: xc first, then resident A) ----
            x_sb = persist.tile([128, NT, D], F32, name="x_sb", tag="bigA")
            nc.sync.dma_start(x_sb[:], xc.ap().rearrange("t p d -> p t d"))

            ident = const.tile([128, 128], F32, name="ident")
            make_identity(nc, ident)
            w_sb = []
            for li, wd in enumerate((w1t, w2t)):
                w = const.tile([128, 2, D], F32, name=f"w{li}")
                nc.sync.dma_start(w[:], wd.ap().rearrange("k p n -> p k n"))
                w_sb.append(w)
            hb_sb = []
            for li, hd in enumerate((hb1, hb2)):
                h = const.tile([128, D], F32, name=f"hb{li}")
                nc.sync.dma_start(h[:], hd.ap())
                hb_sb.append(h)
            u_sb = []
            for li, ud in enumerate((u1, u2)):
                u = const.tile([128, D], F32, name=f"u{li}")
                nc.sync.dma_start(u[:], ud.ap())
                u_sb.append(u)

            # resident half of A: [128, RESN, PC] fp16
            a_res = None
            if RESN:
                a_res = const.tile([128, RESN, PC], F16, name="a_res")
                for s4 in range(4):
                    sl = slice(s4 * (RESN // 4), (s4 + 1) * (RESN // 4))
                    nc.sync.dma_start(a_res[:, sl, :],
                                      ares_d.ap()[sl].rearrange("k p n -> p k n"))

            xt_full = persist.tile([128, KT, D], F16, name="xt_full",
                                   tag="xt_full")
            xtf_view = xt_full[:].rearrange("p (c t) d -> p c t d", t=NT)

            def sc(name):
                return persist.tile([128, NT], F32, name=name)

            def sumsq(src_ap, accum_ap):
                s = sqp.tile([128, D], F32, name="sq_scr", tag="sq_scr")
                nc.scalar.activation(s[:], src_ap, AF.Square,
                                     accum_out=accum_ap)

            def sumsq_psum(src_ap, accum_ap):
                s = sqp.tile([128, D], F32, name="sqp_scr", tag="sqp_scr")
                nc.scalar.activation(s[:], src_ap, AF.Square,
                                     accum_out=accum_ap)

            def clamp_recip(dst, src, name):
                c = sc(name + "_c")
                nc.vector.tensor_scalar_max(c[:], src[:], MINN)
                nc.vector.reciprocal(dst[:], c[:])

            def artanh_ln(dst, x, name):
                """dst = ln((1+x)/(1-x)); caller owns the 0.5 factor."""
                ap1 = sc(name + "_ap")
                am1 = sc(name + "_am")
                ram = sc(name + "_ram")
                q = sc(name + "_q")
                nc.vector.tensor_scalar_add(ap1[:], x[:], 1.0)
                nc.vector.tensor_scalar(am1[:], x[:], -1.0, 1.0,
                                        ALU.mult, ALU.add)
                nc.vector.reciprocal(ram[:], am1[:])
                nc.vector.tensor_tensor(q[:], ap1[:], ram[:], ALU.mult)
                nc.scalar.activation(dst[:], q[:], AF.Ln)

            # ---------------- encode: h = proj(expmap0(x)) ----------------
            h_all = persist.tile([128, NT, D], F32, name="h_all", tag="bigB")
            xn2 = sc("xn2")
            for t in range(NT):
                sumsq(x_sb[:, t, :], xn2[:, t : t + 1])
            un = sc("un")
            nc.scalar.activation(un[:], xn2[:], AF.Sqrt)
            run_ = sc("run")
            clamp_recip(run_, un, "enc_r")
            thx = sc("thx")
            nc.scalar.activation(thx[:], un[:], AF.Tanh)
            mn0 = sc("mn0")
            nc.vector.tensor_scalar_min(mn0[:], thx[:], MAXN)
            s0 = sc("s0")
            nc.vector.tensor_tensor(s0[:], mn0[:], run_[:], ALU.mult)
            for t in range(NT):
                nc.vector.tensor_scalar_mul(h_all[:, t, :], x_sb[:, t, :],
                                            s0[:, t : t + 1])

            def layer(li, h_in, hnorm, e_out_d):
                """One HGCN layer; h_in [128,NT,D] on-ball, hnorm [128,NT] its
                row norms. Returns (e_all, out_norms)."""
                L = f"l{li}_"
                w = w_sb[li]
                hb = hb_sb[li]
                u = u_sb[li]
                y2 = float(y2s[li])

                # ---- ryp = mx.hb = h.(W^T hb), independent of matmuls ----
                ryp = sc(L + "ryp")
                for t in range(NT):
                    s = sqp.tile([128, D], F32, name="ryp_scr", tag="ryp_scr")
                    nc.vector.tensor_tensor(s[:], h_in[:, t, :], u[:],
                                            ALU.mult)
                    nc.vector.reduce_sum(ryp[:, t : t + 1], s[:],
                                         axis=mybir.AxisListType.X)

                # ---- HypLinear matmuls + |mx|^2 ----
                mx_all = persist.tile([128, NT, D], F32, name=L + "mx",
                                      tag="bigA")
                mn2 = sc(L + "mn2")
                for t in range(NT):
                    hT = htp.tile([128, 2, 128], F32, name="hT", tag="hT")
                    for kc in range(2):
                        psT = pst.tile([128, 128], F32, name="psT", tag="psT")
                        nc.tensor.transpose(
                            psT[:], h_in[:, t, kc * 128 : (kc + 1) * 128],
                            ident[:])
                        nc.vector.tensor_copy(hT[:, kc, :], psT[:])
                    pmx = psmx.tile([128, D], F32, name="pmx", tag="pmx")
                    nc.tensor.matmul(pmx[:], hT[:, 0, :], w[:, 0, :],
                                     start=True, stop=False)
                    nc.tensor.matmul(pmx[:], hT[:, 1, :], w[:, 1, :],
                                     start=False, stop=True)
                    nc.scalar.activation(mx_all[:, t, :], pmx[:], AF.Copy)
                    sumsq(mx_all[:, t, :], mn2[:, t : t + 1])

                # ---- SB1: mobius_matvec scalars ----
                mxn = sc(L + "mxn")
                nc.scalar.activation(mxn[:], mn2[:], AF.Sqrt)
                nc.vector.tensor_scalar_max(mxn[:], mxn[:], MINN)
                rxn = sc(L + "rxn")
                clamp_recip(rxn, hnorm, L + "rxn")
                rmxn = sc(L + "rmxn")
                nc.vector.reciprocal(rmxn[:], mxn[:])
                atx = sc(L + "atx")
                artanh_ln(atx, hnorm, L + "atx")
                targ = sc(L + "targ")
                nc.vector.tensor_tensor(targ[:], mxn[:], rxn[:], ALU.mult)
                nc.vector.tensor_tensor(targ[:], targ[:], atx[:], ALU.mult)
                th = sc(L + "th")
                nc.scalar.activation(th[:], targ[:], AF.Tanh, scale=0.5)
                sres = sc(L + "sres")
                nc.vector.tensor_tensor(sres[:], th[:], rmxn[:], ALU.mult)
                # proj of res: norm is th (analytically); f1 = min(MAXN/th, 1)
                rth = sc(L + "rth")
                clamp_recip(rth, th, L + "rth")
                f1 = sc(L + "f1")
                nc.vector.tensor_scalar(f1[:], rth[:], MAXN, 1.0,
                                        ALU.mult, ALU.min)
                nres = sc(L + "nres")
                nc.vector.tensor_scalar_min(nres[:], th[:], MAXN)
                x2 = sc(L + "x2")
                nc.vector.tensor_tensor(x2[:], nres[:], nres[:], ALU.mult)

                # ---- SB2: mobius_add coefficients ----
                xy = sc(L + "xy")
                nc.vector.tensor_tensor(xy[:], ryp[:], sres[:], ALU.mult)
                nc.vector.tensor_tensor(xy[:], xy[:], f1[:], ALU.mult)
                apre = sc(L + "apre")
                nc.vector.tensor_scalar(apre[:], xy[:], 2.0, 1.0 + y2,
                                        ALU.mult, ALU.add)
                alpha = sc(L + "alpha")
                nc.vector.tensor_tensor(alpha[:], apre[:], f1[:], ALU.mult)
                beta = sc(L + "beta")
                nc.vector.tensor_scalar(beta[:], x2[:], -1.0, 1.0,
                                        ALU.mult, ALU.add)
                den = sc(L + "den")
                nc.vector.tensor_scalar(den[:], x2[:], y2, 1.0,
                                        ALU.mult, ALU.add)
                xy2 = sc(L + "xy2")
                nc.vector.tensor_scalar_mul(xy2[:], xy[:], 2.0)
                nc.vector.tensor_tensor(den[:], den[:], xy2[:], ALU.add)
                dinv = sc(L + "dinv")
                clamp_recip(dinv, den, L + "dinv")
                asc = sc(L + "asc")
                nc.vector.tensor_tensor(asc[:], alpha[:], dinv[:], ALU.mult)
                nc.vector.tensor_tensor(asc[:], asc[:], sres[:], ALU.mult)
                bsc = sc(L + "bsc")
                nc.vector.tensor_tensor(bsc[:], beta[:], dinv[:], ALU.mult)

                # ---- |h2|^2 analytically (h2 = asc*mx + bsc*hb) ----
                hn2 = sc(L + "hn2")
                a2 = sc(L + "a2")
                nc.vector.tensor_tensor(a2[:], asc[:], asc[:], ALU.mult)
                nc.vector.tensor_tensor(hn2[:], a2[:], mn2[:], ALU.mult)
                ab = sc(L + "ab")
                nc.vector.tensor_tensor(ab[:], asc[:], bsc[:], ALU.mult)
                abry = sc(L + "abry")
                nc.vector.tensor_tensor(abry[:], ab[:], ryp[:], ALU.mult)
                nc.vector.tensor_scalar_mul(abry[:], abry[:], 2.0)
                nc.vector.tensor_tensor(hn2[:], hn2[:], abry[:], ALU.add)
                b2 = sc(L + "b2")
                nc.vector.tensor_tensor(b2[:], bsc[:], bsc[:], ALU.mult)
                nc.vector.tensor_scalar_mul(b2[:], b2[:], y2)
                nc.vector.tensor_tensor(hn2[:], hn2[:], b2[:], ALU.add)

                # ---- SB3: proj + logmap0 scale ----
                hn = sc(L + "hn")
                nc.scalar.activation(hn[:], hn2[:], AF.Sqrt)
                rhn = sc(L + "rhn")
                clamp_recip(rhn, hn, L + "rhn")
                f2 = sc(L + "f2")
                nc.vector.tensor_scalar(f2[:], rhn[:], MAXN, 1.0,
                                        ALU.mult, ALU.min)
                m = sc(L + "m")
                nc.vector.tensor_scalar_min(m[:], hn[:], MAXN)
                rm = sc(L + "rm")
                clamp_recip(rm, m, L + "rm")
                atm = sc(L + "atm")
                artanh_ln(atm, m, L + "atm")
                g0 = sc(L + "g0")
                nc.vector.tensor_tensor(g0[:], atm[:], rm[:], ALU.mult)
                g = sc(L + "g")
                nc.vector.tensor_tensor(g[:], g0[:], f2[:], ALU.mult)
                nc.vector.tensor_scalar_mul(g[:], g[:], 0.5)
                ga = sc(L + "ga")
                nc.vector.tensor_tensor(ga[:], g[:], asc[:], ALU.mult)
                gb = sc(L + "gb")
                nc.vector.tensor_tensor(gb[:], g[:], bsc[:], ALU.mult)

                # ---- xt = ga*mx + gb*hb (tangent features, fp16) ----
                xt_all = persist.tile([128, NT, D], F16, name=L + "xt",
                                      tag="bigC")
                for t in range(NT):
                    t2 = sqp.tile([128, D], F32, name="t2t", tag="t2t")
                    nc.vector.tensor_scalar_mul(t2[:], hb[:],
                                                gb[:, t : t + 1])
                    t1 = sqp.tile([128, D], F32, name="t1t", tag="t1t")
                    nc.vector.tensor_scalar_mul(t1[:], mx_all[:, t, :],
                                                ga[:, t : t + 1])
                    nc.vector.tensor_tensor(xt_all[:, t, :], t1[:], t2[:],
                                            ALU.add)

                # ---- AllGather tangent features (2 big chunks) ----
                # bounce DMAs ride the scalar HWDGE ring so the sync ring
                # stays dedicated to the A stream.
                ag_backs = []
                for j in range(NCH):
                    agin = dram.tile([TPC, 128, D], F16, name=f"{L}agin{j}",
                                     tag=f"agin{j}")
                    agout = dram.tile([NCORES * TPC, 128, D], F16,
                                      name=f"{L}agout{j}", tag=f"agout{j}",
                                      addr_space="Shared")
                    nc.sync.dma_start(agin[:].rearrange("t p d -> p t d"),
                                        xt_all[:, j * TPC : (j + 1) * TPC, :])
                    nc.gpsimd.collective_compute(
                        "AllGather", ALU.bypass,
                        replica_groups=[list(range(NCORES))],
                        ins=[agin[:].opt()], outs=[agout[:].opt()])
                    agout_v = agout[:].rearrange("(c t) p d -> p c t d", t=TPC)
                    for i in range(TPC):
                        nc.sync.dma_start(
                            xtf_view[:, :, j * TPC + i, :],
                            agout_v[:, :, i, :])

                # ---- spmm: agg[dst, f] = sum_src AT[src, dst] xt[src, f] ----
                # chunk-0 k-tiles come from resident SBUF, chunk-1 streams
                # from HBM (sync ring). PSUM 'start' clears the whole 2KB
                # bank; tiles t, t+1 share a bank, so only the very first
                # matmul of each even tile issues start=True.
                pagg = psagg.tile([128, NT, D], F32, name="pagg", tag="pagg")
                k_iter = 0
                for j in range(NCH):
                    for i in range(TPC):
                        for c in range(NCORES):
                            kt = c * NT + j * TPC + i
                            s = (j * TPC + i) * NCORES + c
                            if s < RESN:
                                lhs = a_res[:, s, :]
                            else:
                                at_k = atp.tile([128, PC], F16, name="at_k",
                                                tag="at_k")
                                nc.sync.dma_start(at_k[:],
                                                  astr_d.ap()[s - RESN])
                                lhs = at_k[:]
                            for t in range(NT):
                                nc.tensor.matmul(
                                    pagg[:, t, :],
                                    lhs[:, t * 128 : (t + 1) * 128],
                                    xt_full[:, kt, :],
                                    start=(k_iter == 0 and t % 2 == 0),
                                    stop=(k_iter == KT - 1),
                                    skip_group_check=True)
                            k_iter += 1

                # ---- HypAct ----
                r2 = sc(L + "r2")
                for t in range(NT):
                    sumsq_psum(pagg[:, t, :], r2[:, t : t + 1])
                rn = sc(L + "rn")
                nc.scalar.activation(rn[:], r2[:], AF.Sqrt)
                rrn = sc(L + "rrn")
                clamp_recip(rrn, rn, L + "rrn")
                th2 = sc(L + "th2")
                nc.scalar.activation(th2[:], rn[:], AF.Tanh)
                m1 = sc(L + "m1")
                nc.vector.tensor_scalar_min(m1[:], th2[:], MAXN)
                rm1 = sc(L + "rm1")
                clamp_recip(rm1, m1, L + "rm1")
                s1 = sc(L + "s1")
                nc.vector.tensor_tensor(s1[:], m1[:], rrn[:], ALU.mult)
                atq = sc(L + "atq")
                artanh_ln(atq, m1, L + "atq")
                qs0 = sc(L + "qs0")
                nc.vector.tensor_tensor(qs0[:], s1[:], atq[:], ALU.mult)
                qs = sc(L + "qs")
                nc.vector.tensor_tensor(qs[:], qs0[:], rm1[:], ALU.mult)
                nc.vector.tensor_scalar_mul(qs[:], qs[:], 0.5)

                xt2_all = persist.tile([128, NT, D], F32, name=L + "xt2",
                                       tag="bigD")
                n2b = sc(L + "n2b")
                for t in range(NT):
                    # relu(qs*agg) on DVE (PSUM src)
                    nc.vector.tensor_scalar(xt2_all[:, t, :], pagg[:, t, :],
                                            qs[:, t : t + 1], 0.0,
                                            ALU.mult, ALU.max)
                    sumsq(xt2_all[:, t, :], n2b[:, t : t + 1])

                un2 = sc(L + "un2")
                nc.scalar.activation(un2[:], n2b[:], AF.Sqrt)
                run2 = sc(L + "run2")
                clamp_recip(run2, un2, L + "run2")
                th3 = sc(L + "th3")
                nc.scalar.activation(th3[:], un2[:], AF.Tanh)
                mm2 = sc(L + "mm2")
                nc.vector.tensor_scalar_min(mm2[:], th3[:], MAXN)
                ss = sc(L + "ss")
                nc.vector.tensor_tensor(ss[:], mm2[:], run2[:], ALU.mult)

                e_all = persist.tile([128, NT, D], F32, name=L + "e",
                                     tag="bigB")
                for t in range(NT):
                    nc.vector.tensor_scalar_mul(e_all[:, t, :],
                                                xt2_all[:, t, :],
                                                ss[:, t : t + 1])
                nc.sync.dma_start(e_out_d.ap().rearrange("t p d -> p t d"),
                                    e_all[:])
                return e_all, mm2

            e1_all, n1 = layer(0, h_all, mn0, e1_d)
            layer(1, e1_all, n1, e2_d)

    nc.compile()
    return nc


def _hyp_bias(b):
    """proj(expmap0(b, c=1), c=1) in float32, mirroring the reference."""
    b = b.astype(np.float32)
    un = np.maximum(np.sqrt((b * b).sum()), np.float32(MINN)).astype(np.float32)
    h = (np.tanh(un) * b / un).astype(np.float32)
    n = np.maximum(np.sqrt((h * h).sum()), np.float32(MINN)).astype(np.float32)
    if n > np.float32(MAXN):
        h = (h / n * np.float32(MAXN)).astype(np.float32)
    return h


def prepare_inputs(x, W1, b1, W2, b2, edge_index, edge_weight):
    x = np.asarray(x, np.float32)
    W1 = np.asarray(W1, np.float32)
    W2 = np.asarray(W2, np.float32)
    b1 = np.asarray(b1, np.float32)
    b2 = np.asarray(b2, np.float32)
    ew = np.asarray(edge_weight, np.float32)
    src = np.asarray(edge_index[0], np.int64)
    dst = np.asarray(edge_index[1], np.int64)

    AT = np.zeros((NP, NP), np.float32)
    np.add.at(AT, (src, dst), ew)

    xfull = np.zeros((NP, D), np.float32)
    xfull[:N] = x

    hb1 = _hyp_bias(b1)
    hb2 = _hyp_bias(b2)
    y2s = (float((hb1.astype(np.float64) ** 2).sum()),
           float((hb2.astype(np.float64) ** 2).sum()))
    u1 = (W1.T.astype(np.float64) @ hb1.astype(np.float64)).astype(np.float32)
    u2 = (W2.T.astype(np.float64) @ hb2.astype(np.float64)).astype(np.float32)

    w1t = np.ascontiguousarray(W1.T).reshape(2, 128, D)
    w2t = np.ascontiguousarray(W2.T).reshape(2, 128, D)
    hb1_b = np.tile(hb1[None, :], (128, 1)).astype(np.float32)
    hb2_b = np.tile(hb2[None, :], (128, 1)).astype(np.float32)
    u1_b = np.tile(u1[None, :], (128, 1)).astype(np.float32)
    u2_b = np.tile(u2[None, :], (128, 1)).astype(np.float32)

    # consumption order: slot s = (j*TPC + i)*NCORES + c, kt = c*NT + j*TPC+i
    # first RESN slots live in SBUF, the rest stream per layer
    in_maps = []
    for core in range(NCORES):
        ac = np.ascontiguousarray(
            AT[:, core * PC : (core + 1) * PC]).reshape(KT, 128, PC)
        ac = ac.astype(np.float16)
        a_all = np.empty((KT, 128, PC), np.float16)
        for j in range(NCH):
            for i in range(TPC):
                for c in range(NCORES):
                    a_all[(j * TPC + i) * NCORES + c] = ac[c * NT + j * TPC + i]
        a_res = np.ascontiguousarray(a_all[:max(RESN, 1)])
        a_str = np.ascontiguousarray(a_all[RESN:])
        xcr = xfull[core * PC : (core + 1) * PC].reshape(NT, 128, D)
        in_maps.append({
            "xc": np.ascontiguousarray(xcr),
            "ares": a_res,
            "astr": a_str,
            "w1t": w1t, "w2t": w2t,
            "hb1": hb1_b, "hb2": hb2_b,
            "u1": u1_b, "u2": u2_b,
        })
    return in_maps, y2s


def assemble(results):
    e1 = np.concatenate([r["e1"].reshape(PC, D) for r in results], 0)[:N]
    e2 = np.concatenate([r["e2"].reshape(PC, D) for r in results], 0)[:N]
    return np.stack([e1, e2], 0).astype(np.float32)


def run(inputs, trace=False):
    in_maps, y2s = prepare_inputs(**inputs)
    nc = build_nc(y2s)
    res = run_bass_kernel_spmd(nc, in_maps, core_ids=list(range(NCORES)),
                               trace=trace)
    return assemble(res.results), res


def kernel(**inputs):
    out, _ = run(inputs, trace=False)
    return out


# revision 27
# speedup vs baseline: 1.1426x; 1.0557x over previous
"""HGCN (hyperbolic GCN) 2-layer forward for Trainium2, 8 NeuronCores.

Strategy (graph-parallel, dense-spmm):
  - Nodes padded 10000 -> 10240 and sharded 1280/core (8 cores).
  - segment_sum over the edge list is recast as a dense matmul
    agg = A @ xt with A[dst, src] = sum of edge weights; A is built on the
    host and each core owns its [10240 x 1280] column slice in fp16.
  - 32 of the 80 A k-tiles stay resident in SBUF across both layers; the
    rest stream from HBM during the spmm.
  - expmap0/proj scaling is per-node, and matmul is linear, so HypLinear
    runs on RAW tangent vectors (x, or the previous layer's relu output)
    and all on-manifold scaling folds into [128, 10] scalar chains.  The
    PE therefore starts each layer's matmuls before the previous chain
    finishes, and the final e = ss*xt2 materialization is off the
    critical path (only feeds the output DMA).
  - Tangent features are AllGathered in 2 big fp16 chunks (the ~10us
    collective floor amortizes); the spmm runs 80 k-tiles back-to-back
    to keep the PE at max p-state.
  - Scalar-engine activation tables (Sqrt/Tanh/Ln/Square) are prewarmed
    with dummy ops so the ~1.3us table load hides under DVE chain work.

kernel(**inputs) takes the FULL unsharded inputs and returns [2, N, D].
"""

import sys

import numpy as np

for _p in ("/opt/trn_rl_repo",):
    if _p not in sys.path:
        sys.path.append(_p)

import concourse.tile as tile  # noqa: E402
from concourse import bacc, mybir  # noqa: E402
from concourse.bass_utils import run_bass_kernel_spmd  # noqa: E402
from concourse.masks import make_identity  # noqa: E402

AF = mybir.ActivationFunctionType
ALU = mybir.AluOpType
AX = mybir.AxisListType
F32 = mybir.dt.float32
F16 = mybir.dt.float16

NCORES = 8
N = 10000
D = 256
NP = 10240
PC = NP // NCORES      # 1280 nodes per core
NT = PC // 128         # 10 node tiles per core
KT = NP // 128         # 80 contraction tiles
NCH = 2                # AllGather chunks per layer
TPC = NT // NCH        # local tiles per AG chunk (5)
RESN = 32              # resident k-tiles (SBUF), rest stream per layer
MAXN = 1.0 - 4e-3      # PROJ_EPS clip for c=1
MINN = 1e-15


def build_nc(y2s):
    """Build the per-core Bass program. y2s = (||hyp_b1||^2, ||hyp_b2||^2)."""
    nc = bacc.Bacc("TRN2", target_bir_lowering=False, debug=False,
                   num_devices=NCORES)

    xc = nc.dram_tensor("xc", [NT, 128, D], F32, kind="ExternalInput")
    ares_d = nc.dram_tensor("ares", [max(RESN, 1), 128, PC], F16,
                            kind="ExternalInput")
    astr_d = nc.dram_tensor("astr", [KT - RESN, 128, PC], F16,
                            kind="ExternalInput")
    w1t = nc.dram_tensor("w1t", [2, 128, D], F32, kind="ExternalInput")
    w2t = nc.dram_tensor("w2t", [2, 128, D], F32, kind="ExternalInput")
    hb1 = nc.dram_tensor("hb1", [128, D], F32, kind="ExternalInput")
    hb2 = nc.dram_tensor("hb2", [128, D], F32, kind="ExternalInput")
    u1 = nc.dram_tensor("u1", [128, D], F32, kind="ExternalInput")
    u2 = nc.dram_tensor("u2", [128, D], F32, kind="ExternalInput")
    e1_d = nc.dram_tensor("e1", [NT, 128, D], F32, kind="ExternalOutput")
    e2_d = nc.dram_tensor("e2", [NT, 128, D], F32, kind="ExternalOutput")

    with tile.TileContext(nc) as tc:
        with (
            tc.tile_pool(name="const", bufs=1) as const,
            tc.tile_pool(name="persist", bufs=1) as persist,
            tc.tile_pool(name="sqp", bufs=4) as sqp,
            tc.tile_pool(name="htp", bufs=2) as htp,
            tc.tile_pool(name="atp", bufs=4) as atp,
            tc.tile_pool(name="pst", bufs=2, space="PSUM") as pst,
            tc.tile_pool(name="psmx", bufs=1, space="PSUM") as psmx,
            tc.tile_pool(name="psagg", bufs=1, space="PSUM") as psagg,
            tc.tile_pool(name="dram", bufs=1, space="DRAM") as dram,
        ):
            # ---- const loads (sync ring: xc first, then resident A) ----
            x_sb = persist.tile([128, NT, D], F32, name="x_sb", tag="bigA")
            nc.sync.dma_start(x_sb[:], xc.ap().rearrange("t p d -> p t d"))

            ident = const.tile([128, 128], F32, name="ident")
            make_identity(nc, ident)
            w_sb = []
            for li, wd in enumerate((w1t, w2t)):
                w = const.tile([128, 2, D], F32, name=f"w{li}")
                nc.sync.dma_start(w[:], wd.ap().rearrange("k p n -> p k n"))
                w_sb.append(w)
            hb_sb = []
            for li, hd in enumerate((hb1, hb2)):
                h = const.tile([128, D], F32, name=f"hb{li}")
                nc.sync.dma_start(h[:], hd.ap())
                hb_sb.append(h)
            u_sb = []
            for li, ud in enumerate((u1, u2)):
                u = const.tile([128, D], F32, name=f"u{li}")
                nc.sync.dma_start(u[:], ud.ap())
                u_sb.append(u)

            # resident part of A: [128, RESN, PC] fp16, ~80 KB/partition
            a_res = None
            if RESN:
                a_res = const.tile([128, RESN, PC], F16, name="a_res")
                for s4 in range(4):
                    sl = slice(s4 * (RESN // 4), (s4 + 1) * (RESN // 4))
                    nc.sync.dma_start(
                        a_res[:, sl, :],
                        ares_d.ap()[sl].rearrange("k p n -> p k n"))

            xt_full = persist.tile([128, KT, D], F16, name="xt_full",
                                   tag="xt_full")
            xtf_view = xt_full[:].rearrange("p (c t) d -> p c t d", t=NT)

            # activation-table prewarm scratch
            wsrc = const.tile([128, 1], F32, name="wsrc")
            nc.vector.memset(wsrc[:], 0.5)
            warm = const.tile([128, 1], F32, name="warm")

            def prewarm(func):
                nc.scalar.activation(warm[:], wsrc[:], func)

            def sc(name):
                return persist.tile([128, NT], F32, name=name)

            def sumsq(src_ap, accum_ap):
                # SBUF src: square+reduce on DVE
                s = sqp.tile([128, D], F32, name="sq_scr", tag="sq_scr")
                nc.vector.tensor_tensor(s[:], src_ap, src_ap, ALU.mult)
                nc.vector.reduce_sum(accum_ap, s[:], axis=AX.X)

            def sumsq_ps(src_ap, accum_ap):
                # PSUM src: Square+accum on the scalar engine
                s = sqp.tile([128, D], F32, name="sqp_scr", tag="sqp_scr")
                nc.scalar.activation(s[:], src_ap, AF.Square,
                                     accum_out=accum_ap)

            def clamp_recip(dst, src, name):
                c = sc(name + "_c")
                nc.vector.tensor_scalar_max(c[:], src[:], MINN)
                nc.vector.reciprocal(dst[:], c[:])

            def artanh_ln(dst, x, name):
                """dst = ln((1+x)/(1-x)); caller owns the 0.5 factor."""
                ap1 = sc(name + "_ap")
                am1 = sc(name + "_am")
                ram = sc(name + "_ram")
                q = sc(name + "_q")
                nc.vector.tensor_scalar_add(ap1[:], x[:], 1.0)
                nc.vector.tensor_scalar(am1[:], x[:], -1.0, 1.0,
                                        ALU.mult, ALU.add)
                nc.vector.reciprocal(ram[:], am1[:])
                nc.vector.tensor_tensor(q[:], ap1[:], ram[:], ALU.mult)
                nc.scalar.activation(dst[:], q[:], AF.Ln)

            # -------- encode scalars: s0 = min(tanh|x|,MAXN)/|x| ----------
            # h = proj(expmap0(x)) = s0 * x is never materialized; s0 folds
            # into the layer's scalar chains.
            prewarm(AF.Sqrt)
            xn2 = sc("xn2")
            for t in range(NT):
                sumsq(x_sb[:, t, :], xn2[:, t : t + 1])
            un = sc("un")
            nc.scalar.activation(un[:], xn2[:], AF.Sqrt)
            prewarm(AF.Tanh)
            run_ = sc("run")
            clamp_recip(run_, un, "enc_r")
            thx = sc("thx")
            nc.scalar.activation(thx[:], un[:], AF.Tanh)
            prewarm(AF.Sqrt)
            mn0 = sc("mn0")
            nc.vector.tensor_scalar_min(mn0[:], thx[:], MAXN)
            s0 = sc("s0")
            nc.vector.tensor_tensor(s0[:], mn0[:], run_[:], ALU.mult)

            def layer(li, v_in, s_in, hnorm, e_out_d, e_tag):
                """One HGCN layer on raw tangent input: the on-ball h is
                s_in * v_in with |h| = hnorm. Returns (xt2, ss, mm2) where
                the next layer's input is v=xt2 scaled by s=ss."""
                L = f"l{li}_"
                w = w_sb[li]
                hb = hb_sb[li]
                u = u_sb[li]
                y2 = float(y2s[li])

                # ---- raw ryp = v.(W^T hb); scaled later ----
                rypr = sc(L + "rypr")
                for t in range(NT):
                    s = sqp.tile([128, D], F32, name="ryp_scr", tag="ryp_scr")
                    nc.vector.tensor_tensor(s[:], v_in[:, t, :], u[:],
                                            ALU.mult)
                    nc.vector.reduce_sum(rypr[:, t : t + 1], s[:], axis=AX.X)

                # ---- HypLinear matmuls on raw v + |mxr|^2 ----
                mxr_all = persist.tile([128, NT, D], F32, name=L + "mx",
                                       tag="bigB")
                mn2r = sc(L + "mn2r")
                for t in range(NT):
                    hT = htp.tile([128, 2, 128], F32, name="hT", tag="hT")
                    for kc in range(2):
                        psT = pst.tile([128, 128], F32, name="psT", tag="psT")
                        nc.tensor.transpose(
                            psT[:], v_in[:, t, kc * 128 : (kc + 1) * 128],
                            ident[:])
                        nc.vector.tensor_copy(hT[:, kc, :], psT[:])
                    pmx = psmx.tile([128, D], F32, name="pmx", tag="pmx")
                    nc.tensor.matmul(pmx[:], hT[:, 0, :], w[:, 0, :],
                                     start=True, stop=False)
                    nc.tensor.matmul(pmx[:], hT[:, 1, :], w[:, 1, :],
                                     start=False, stop=True)
                    nc.scalar.activation(mxr_all[:, t, :], pmx[:], AF.Copy)
                    sumsq(mxr_all[:, t, :], mn2r[:, t : t + 1])

                # ---- scale bookkeeping: mx = s_in*mxr ----
                s2 = sc(L + "s2")
                nc.vector.tensor_tensor(s2[:], s_in[:], s_in[:], ALU.mult)
                mn2 = sc(L + "mn2")
                nc.vector.tensor_tensor(mn2[:], mn2r[:], s2[:], ALU.mult)
                ryp = sc(L + "ryp")
                nc.vector.tensor_tensor(ryp[:], rypr[:], s_in[:], ALU.mult)

                # ---- SB1: mobius_matvec scalars ----
                mxn = sc(L + "mxn")
                nc.scalar.activation(mxn[:], mn2[:], AF.Sqrt)
                prewarm(AF.Ln)
                nc.vector.tensor_scalar_max(mxn[:], mxn[:], MINN)
                rxn = sc(L + "rxn")
                clamp_recip(rxn, hnorm, L + "rxn")
                rmxn = sc(L + "rmxn")
                nc.vector.reciprocal(rmxn[:], mxn[:])
                atx = sc(L + "atx")
                artanh_ln(atx, hnorm, L + "atx")
                prewarm(AF.Tanh)
                targ = sc(L + "targ")
                nc.vector.tensor_tensor(targ[:], mxn[:], rxn[:], ALU.mult)
                nc.vector.tensor_tensor(targ[:], targ[:], atx[:], ALU.mult)
                th = sc(L + "th")
                nc.scalar.activation(th[:], targ[:], AF.Tanh, scale=0.5)
                prewarm(AF.Sqrt)
                sres = sc(L + "sres")
                nc.vector.tensor_tensor(sres[:], th[:], rmxn[:], ALU.mult)
                # proj of res: norm is th (analytically); f1 = min(MAXN/th, 1)
                rth = sc(L + "rth")
                clamp_recip(rth, th, L + "rth")
                f1 = sc(L + "f1")
                nc.vector.tensor_scalar(f1[:], rth[:], MAXN, 1.0,
                                        ALU.mult, ALU.min)
                nres = sc(L + "nres")
                nc.vector.tensor_scalar_min(nres[:], th[:], MAXN)
                x2 = sc(L + "x2")
                nc.vector.tensor_tensor(x2[:], nres[:], nres[:], ALU.mult)

                # ---- SB2: mobius_add coefficients ----
                xy = sc(L + "xy")
                nc.vector.tensor_tensor(xy[:], ryp[:], sres[:], ALU.mult)
                nc.vector.tensor_tensor(xy[:], xy[:], f1[:], ALU.mult)
                apre = sc(L + "apre")
                nc.vector.tensor_scalar(apre[:], xy[:], 2.0, 1.0 + y2,
                                        ALU.mult, ALU.add)
                alpha = sc(L + "alpha")
                nc.vector.tensor_tensor(alpha[:], apre[:], f1[:], ALU.mult)
                beta = sc(L + "beta")
                nc.vector.tensor_scalar(beta[:], x2[:], -1.0, 1.0,
                                        ALU.mult, ALU.add)
                den = sc(L + "den")
                nc.vector.tensor_scalar(den[:], x2[:], y2, 1.0,
                                        ALU.mult, ALU.add)
                xy2 = sc(L + "xy2")
                nc.vector.tensor_scalar_mul(xy2[:], xy[:], 2.0)
                nc.vector.tensor_tensor(den[:], den[:], xy2[:], ALU.add)
                dinv = sc(L + "dinv")
                clamp_recip(dinv, den, L + "dinv")
                asc = sc(L + "asc")
                nc.vector.tensor_tensor(asc[:], alpha[:], dinv[:], ALU.mult)
                nc.vector.tensor_tensor(asc[:], asc[:], sres[:], ALU.mult)
                bsc = sc(L + "bsc")
                nc.vector.tensor_tensor(bsc[:], beta[:], dinv[:], ALU.mult)

                # ---- |h2|^2 analytically (h2 = asc*mx + bsc*hb) ----
                hn2 = sc(L + "hn2")
                a2 = sc(L + "a2")
                nc.vector.tensor_tensor(a2[:], asc[:], asc[:], ALU.mult)
                nc.vector.tensor_tensor(hn2[:], a2[:], mn2[:], ALU.mult)
                ab = sc(L + "ab")
                nc.vector.tensor_tensor(ab[:], asc[:], bsc[:], ALU.mult)
                abry = sc(L + "abry")
                nc.vector.tensor_tensor(abry[:], ab[:], ryp[:], ALU.mult)
                nc.vector.tensor_scalar_mul(abry[:], abry[:], 2.0)
                nc.vector.tensor_tensor(hn2[:], hn2[:], abry[:], ALU.add)
                b2 = sc(L + "b2")
                nc.vector.tensor_tensor(b2[:], bsc[:], bsc[:], ALU.mult)
                nc.vector.tensor_scalar_mul(b2[:], b2[:], y2)
                nc.vector.tensor_tensor(hn2[:], hn2[:], b2[:], ALU.add)

                # ---- SB3: proj + logmap0 scale ----
                hn = sc(L + "hn")
                nc.scalar.activation(hn[:], hn2[:], AF.Sqrt)
                prewarm(AF.Ln)
                rhn = sc(L + "rhn")
                clamp_recip(rhn, hn, L + "rhn")
                f2 = sc(L + "f2")
                nc.vector.tensor_scalar(f2[:], rhn[:], MAXN, 1.0,
                                        ALU.mult, ALU.min)
                m = sc(L + "m")
                nc.vector.tensor_scalar_min(m[:], hn[:], MAXN)
                rm = sc(L + "rm")
                clamp_recip(rm, m, L + "rm")
                atm = sc(L + "atm")
                artanh_ln(atm, m, L + "atm")
                prewarm(AF.Square)
                g0 = sc(L + "g0")
                nc.vector.tensor_tensor(g0[:], atm[:], rm[:], ALU.mult)
                g = sc(L + "g")
                nc.vector.tensor_tensor(g[:], g0[:], f2[:], ALU.mult)
                nc.vector.tensor_scalar_mul(g[:], g[:], 0.5)
                gas = sc(L + "gas")
                nc.vector.tensor_tensor(gas[:], g[:], asc[:], ALU.mult)
                nc.vector.tensor_tensor(gas[:], gas[:], s_in[:], ALU.mult)
                gb = sc(L + "gb")
                nc.vector.tensor_tensor(gb[:], g[:], bsc[:], ALU.mult)

                # ---- xt = gas*mxr + gb*hb (tangent features, fp16) ----
                xt_all = persist.tile([128, NT, D], F16, name=L + "xt",
                                      tag="bigC")
                for t in range(NT):
                    t2 = sqp.tile([128, D], F32, name="t2t", tag="t2t")
                    nc.vector.tensor_scalar_mul(t2[:], hb[:],
                                                gb[:, t : t + 1])
                    t1 = sqp.tile([128, D], F32, name="t1t", tag="t1t")
                    nc.vector.tensor_scalar_mul(t1[:], mxr_all[:, t, :],
                                                gas[:, t : t + 1])
                    nc.vector.tensor_tensor(xt_all[:, t, :], t1[:], t2[:],
                                            ALU.add)

                # ---- AllGather tangent features (2 big chunks) ----
                for j in range(NCH):
                    agin = dram.tile([TPC, 128, D], F16, name=f"{L}agin{j}",
                                     tag=f"agin{j}")
                    agout = dram.tile([NCORES * TPC, 128, D], F16,
                                      name=f"{L}agout{j}", tag=f"agout{j}",
                                      addr_space="Shared")
                    nc.sync.dma_start(agin[:].rearrange("t p d -> p t d"),
                                      xt_all[:, j * TPC : (j + 1) * TPC, :])
                    nc.gpsimd.collective_compute(
                        "AllGather", ALU.bypass,
                        replica_groups=[list(range(NCORES))],
                        ins=[agin[:].opt()], outs=[agout[:].opt()])
                    agout_v = agout[:].rearrange("(c t) p d -> p c t d", t=TPC)
                    for i in range(TPC):
                        nc.sync.dma_start(
                            xtf_view[:, :, j * TPC + i, :],
                            agout_v[:, :, i, :])

                # ---- spmm: agg[dst, f] = sum_src AT[src, dst] xt[src, f] ----
                # i-major so each AG back-DMA (per i, all cores) unblocks 8
                # k-tiles at a time; first RESN slots come from SBUF.
                # PSUM 'start' clears the whole 2KB bank; tiles t, t+1 share
                # a bank, so only the very first matmul of each even tile
                # issues start=True.
                pagg = psagg.tile([128, NT, D], F32, name="pagg", tag="pagg")
                k_iter = 0
                for j in range(NCH):
                    for i in range(TPC):
                        for c in range(NCORES):
                            kt = c * NT + j * TPC + i
                            s = (j * TPC + i) * NCORES + c
                            if s < RESN:
                                lhs = a_res[:, s, :]
                            else:
                                at_k = atp.tile([128, PC], F16, name="at_k",
                                                tag="at_k")
                                nc.sync.dma_start(at_k[:],
                                                  astr_d.ap()[s - RESN])
                                lhs = at_k[:]
                            for t in range(NT):
                                nc.tensor.matmul(
                                    pagg[:, t, :],
                                    lhs[:, t * 128 : (t + 1) * 128],
                                    xt_full[:, kt, :],
                                    start=(k_iter == 0 and t % 2 == 0),
                                    stop=(k_iter == KT - 1),
                                    skip_group_check=True)
                            k_iter += 1

                # ---- HypAct ----
                r2 = sc(L + "r2")
                for t in range(NT):
                    sumsq_ps(pagg[:, t, :], r2[:, t : t + 1])
                rn = sc(L + "rn")
                nc.scalar.activation(rn[:], r2[:], AF.Sqrt)
                prewarm(AF.Tanh)
                rrn = sc(L + "rrn")
                clamp_recip(rrn, rn, L + "rrn")
                th2 = sc(L + "th2")
                nc.scalar.activation(th2[:], rn[:], AF.Tanh)
                prewarm(AF.Ln)
                m1 = sc(L + "m1")
                nc.vector.tensor_scalar_min(m1[:], th2[:], MAXN)
                rm1 = sc(L + "rm1")
                clamp_recip(rm1, m1, L + "rm1")
                s1 = sc(L + "s1")
                nc.vector.tensor_tensor(s1[:], m1[:], rrn[:], ALU.mult)
                atq = sc(L + "atq")
                artanh_ln(atq, m1, L + "atq")
                prewarm(AF.Sqrt)
                qs0 = sc(L + "qs0")
                nc.vector.tensor_tensor(qs0[:], s1[:], atq[:], ALU.mult)
                qs = sc(L + "qs")
                nc.vector.tensor_tensor(qs[:], qs0[:], rm1[:], ALU.mult)
                nc.vector.tensor_scalar_mul(qs[:], qs[:], 0.5)

                xt2_all = persist.tile([128, NT, D], F32, name=L + "xt2",
                                       tag="bigD")
                n2b = sc(L + "n2b")
                for t in range(NT):
                    # relu(qs*agg) on DVE (PSUM src)
                    nc.vector.tensor_scalar(xt2_all[:, t, :], pagg[:, t, :],
                                            qs[:, t : t + 1], 0.0,
                                            ALU.mult, ALU.max)
                    sumsq(xt2_all[:, t, :], n2b[:, t : t + 1])

                un2 = sc(L + "un2")
                nc.scalar.activation(un2[:], n2b[:], AF.Sqrt)
                prewarm(AF.Tanh)
                run2 = sc(L + "run2")
                clamp_recip(run2, un2, L + "run2")
                th3 = sc(L + "th3")
                nc.scalar.activation(th3[:], un2[:], AF.Tanh)
                prewarm(AF.Sqrt)
                mm2 = sc(L + "mm2")
                nc.vector.tensor_scalar_min(mm2[:], th3[:], MAXN)
                ss = sc(L + "ss")
                nc.vector.tensor_tensor(ss[:], mm2[:], run2[:], ALU.mult)

                # e = ss*xt2: off the critical path, only feeds the output
                e_all = persist.tile([128, NT, D], F32, name=L + "e",
                                     tag=e_tag)
                for t in range(NT):
                    nc.vector.tensor_scalar_mul(e_all[:, t, :],
                                                xt2_all[:, t, :],
                                                ss[:, t : t + 1])
                nc.sync.dma_start(e_out_d.ap().rearrange("t p d -> p t d"),
                                  e_all[:])
                return xt2_all, ss, mm2

            xt2_1, ss1, n1 = layer(0, x_sb, s0, mn0, e1_d, "bigA")
            layer(1, xt2_1, ss1, n1, e2_d, "bigA")

    nc.compile()
    return nc


def _hyp_bias(b):
    """proj(expmap0(b, c=1), c=1) in float32, mirroring the reference."""
    b = b.astype(np.float32)
    un = np.maximum(np.sqrt((b * b).sum()), np.float32(MINN)).astype(np.float32)
    h = (np.tanh(un) * b / un).astype(np.float32)
    n = np.maximum(np.sqrt((h * h).sum()), np.float32(MINN)).astype(np.float32)
    if n > np.float32(MAXN):
        h = (h / n * np.float32(MAXN)).astype(np.float32)
    return h


def prepare_inputs(x, W1, b1, W2, b2, edge_index, edge_weight):
    x = np.asarray(x, np.float32)
    W1 = np.asarray(W1, np.float32)
    W2 = np.asarray(W2, np.float32)
    b1 = np.asarray(b1, np.float32)
    b2 = np.asarray(b2, np.float32)
    ew = np.asarray(edge_weight, np.float32)
    src = np.asarray(edge_index[0], np.int64)
    dst = np.asarray(edge_index[1], np.int64)

    AT = np.zeros((NP, NP), np.float32)
    np.add.at(AT, (src, dst), ew)

    xfull = np.zeros((NP, D), np.float32)
    xfull[:N] = x

    hb1 = _hyp_bias(b1)
    hb2 = _hyp_bias(b2)
    y2s = (float((hb1.astype(np.float64) ** 2).sum()),
           float((hb2.astype(np.float64) ** 2).sum()))
    u1 = (W1.T.astype(np.float64) @ hb1.astype(np.float64)).astype(np.float32)
    u2 = (W2.T.astype(np.float64) @ hb2.astype(np.float64)).astype(np.float32)

    w1t = np.ascontiguousarray(W1.T).reshape(2, 128, D)
    w2t = np.ascontiguousarray(W2.T).reshape(2, 128, D)
    hb1_b = np.tile(hb1[None, :], (128, 1)).astype(np.float32)
    hb2_b = np.tile(hb2[None, :], (128, 1)).astype(np.float32)
    u1_b = np.tile(u1[None, :], (128, 1)).astype(np.float32)
    u2_b = np.tile(u2[None, :], (128, 1)).astype(np.float32)

    # consumption order: slot s = (j*TPC + i)*NCORES + c, kt = c*NT + j*TPC+i
    in_maps = []
    for core in range(NCORES):
        ac = np.ascontiguousarray(
            AT[:, core * PC : (core + 1) * PC]).reshape(KT, 128, PC)
        ac = ac.astype(np.float16)
        a_all = np.empty((KT, 128, PC), np.float16)
        for j in range(NCH):
            for i in range(TPC):
                for c in range(NCORES):
                    a_all[(j * TPC + i) * NCORES + c] = ac[c * NT + j * TPC + i]
        a_res = np.ascontiguousarray(a_all[:max(RESN, 1)])
        a_str = np.ascontiguousarray(a_all[RESN:])
        xcr = xfull[core * PC : (core + 1) * PC].reshape(NT, 128, D)
        in_maps.append({
            "xc": np.ascontiguousarray(xcr),
            "ares": a_res,
            "astr": a_str,
            "w1t": w1t, "w2t": w2t,
            "hb1": hb1_b, "hb2": hb2_b,
            "u1": u1_b, "u2": u2_b,
        })
    return in_maps, y2s


def assemble(results):
    e1 = np.concatenate([r["e1"].reshape(PC, D) for r in results], 0)[:N]
    e2 = np.concatenate([r["e2"].reshape(PC, D) for r in results], 0)[:N]
    return np.stack([e1, e2], 0).astype(np.float32)


def run(inputs, trace=False):
    in_maps, y2s = prepare_inputs(**inputs)
    nc = build_nc(y2s)
    res = run_bass_kernel_spmd(nc, in_maps, core_ids=list(range(NCORES)),
                               trace=trace)
    return assemble(res.results), res


def kernel(**inputs):
    out, _ = run(inputs, trace=False)
    return out


# revision 37
# speedup vs baseline: 1.2192x; 1.0670x over previous
"""HGCN (hyperbolic GCN) 2-layer forward for Trainium2, 8 NeuronCores.

Strategy (graph-parallel, dense-spmm):
  - Nodes padded 10000 -> 10240 and sharded 1280/core (8 cores).
  - segment_sum over the edge list is recast as a dense matmul
    agg = A @ xt with A[dst, src] = sum of edge weights; A is built on the
    host and each core owns its [10240 x 1280] column slice in fp16.
  - 32 of the 80 A k-tiles stay resident in SBUF across both layers; the
    rest stream from HBM during the spmm.
  - expmap0/proj scaling is per-node, and matmul is linear, so HypLinear
    runs on RAW tangent vectors (x, or the previous layer's relu output)
    and all on-manifold scaling folds into [128, 10] scalar chains.  The
    PE therefore starts each layer's matmuls before the previous chain
    finishes, and the final e = ss*xt2 materialization is off the
    critical path (only feeds the output DMA).
  - Tangent features are AllGathered in 2 big fp16 chunks (the ~10us
    collective floor amortizes); the spmm runs 80 k-tiles back-to-back
    to keep the PE at max p-state.
  - Scalar-engine activation tables (Sqrt/Tanh/Ln/Square) are prewarmed
    with dummy ops so the ~1.3us table load hides under DVE chain work.

kernel(**inputs) takes the FULL unsharded inputs and returns [2, N, D].
"""

import sys

import numpy as np

for _p in ("/opt/trn_rl_repo",):
    if _p not in sys.path:
        sys.path.append(_p)

import concourse.tile as tile  # noqa: E402
from concourse import bacc, mybir  # noqa: E402

# Pin Ln/Exp/Square to the one activation-table set that holds all three
# (natural_log_exp_and_others): the load-insertion pass then emits a single
# ACT_TABLE_LOAD instead of one per function transition. Other sets only
# lose these entries from the chooser's view; set ids stay positional.
_ORIG_GAT = bacc.get_activation_tables
_PIN_SET = "natural_log_exp_and_others"


def _pinned_gat(arch):
    tabs = _ORIG_GAT(arch)
    pin = {mybir.ActivationFunctionType.Ln, mybir.ActivationFunctionType.Exp,
           mybir.ActivationFunctionType.Square}
    return {name: (fns if name == _PIN_SET else fns - pin)
            for name, fns in tabs.items()}


bacc.get_activation_tables = _pinned_gat
from concourse.bass_utils import run_bass_kernel_spmd  # noqa: E402
from concourse.masks import make_identity  # noqa: E402

AF = mybir.ActivationFunctionType
ALU = mybir.AluOpType
AX = mybir.AxisListType
F32 = mybir.dt.float32
F16 = mybir.dt.float16

NCORES = 8
N = 10000
D = 256
NP = 10240
PC = NP // NCORES      # 1280 nodes per core
NT = PC // 128         # 10 node tiles per core
KT = NP // 128         # 80 contraction tiles
CH = (2, 4, 4)         # AllGather chunk sizes (local tiles per chunk)
CHOFF = (0, 2, 6)      # running offsets of CH
RESN = 32              # resident k-tiles (SBUF), rest stream per layer
PD = 260               # HypLinear psum row: 256 outputs + v.u + pad
MAXN = 1.0 - 4e-3      # PROJ_EPS clip for c=1
MINN = 1e-15


def build_nc(y2s):
    """Build the per-core Bass program. y2s = (||hyp_b1||^2, ||hyp_b2||^2)."""
    nc = bacc.Bacc("TRN2", target_bir_lowering=False, debug=False,
                   num_devices=NCORES)

    xc = nc.dram_tensor("xc", [NT, 128, D], F32, kind="ExternalInput")
    ares_d = nc.dram_tensor("ares", [max(RESN, 1), 128, PC], F16,
                            kind="ExternalInput")
    astr_d = nc.dram_tensor("astr", [KT - RESN, 128, PC], F16,
                            kind="ExternalInput")
    w1t = nc.dram_tensor("w1t", [2, 128, PD], F32, kind="ExternalInput")
    w2t = nc.dram_tensor("w2t", [2, 128, PD], F32, kind="ExternalInput")
    hb1 = nc.dram_tensor("hb1", [128, D], F32, kind="ExternalInput")
    hb2 = nc.dram_tensor("hb2", [128, D], F32, kind="ExternalInput")
    e1_d = nc.dram_tensor("e1", [NT, 128, D], F32, kind="ExternalOutput")
    e2_d = nc.dram_tensor("e2", [NT, 128, D], F32, kind="ExternalOutput")

    with tile.TileContext(nc) as tc:
        with (
            tc.tile_pool(name="const", bufs=1) as const,
            tc.tile_pool(name="persist", bufs=1) as persist,
            tc.tile_pool(name="sqp", bufs=2) as sqp,
            tc.tile_pool(name="htp", bufs=2) as htp,
            tc.tile_pool(name="atp", bufs=7) as atp,
            tc.tile_pool(name="pst", bufs=2, space="PSUM") as pst,
            tc.tile_pool(name="psmx", bufs=1, space="PSUM") as psmx,
            tc.tile_pool(name="psagg", bufs=1, space="PSUM") as psagg,
            tc.tile_pool(name="dram", bufs=1, space="DRAM") as dram,
        ):
            # tiny warm-up AllGather: pays the cross-core barrier + ncfw
            # cold-start during the lead-in instead of before the first
            # real gather; its input DMA rides the (empty) scalar ring
            wag_in = dram.tile([1, 128, 4], F16, name="wag_in", tag="wag_in")
            wag_out = dram.tile([NCORES, 128, 4], F16, name="wag_out",
                                tag="wag_out", addr_space="Shared")
            wag_src = const.tile([128, 4], F16, name="wag_src")
            nc.vector.memset(wag_src[:], 0.0)
            nc.scalar.dma_start(wag_in[:].rearrange("o p d -> p (o d)"),
                                wag_src[:])
            nc.gpsimd.collective_compute(
                "AllGather", ALU.bypass,
                replica_groups=[list(range(NCORES))],
                ins=[wag_in[:].opt()], outs=[wag_out[:].opt()])

            # ---- const loads (sync ring: xc first, then resident A) ----
            x_sb = persist.tile([128, NT, D], F32, name="x_sb", tag="bigA")
            nc.sync.dma_start(x_sb[:], xc.ap().rearrange("t p d -> p t d"))

            ident = const.tile([128, 128], F32, name="ident")
            make_identity(nc, ident)
            w_sb = []
            for li, wd in enumerate((w1t, w2t)):
                w = const.tile([128, 2, PD], F32, name=f"w{li}")
                nc.sync.dma_start(w[:], wd.ap().rearrange("k p n -> p k n"))
                w_sb.append(w)
            hb_sb = []
            for li, hd in enumerate((hb1, hb2)):
                h = const.tile([128, D], F32, name=f"hb{li}")
                nc.sync.dma_start(h[:], hd.ap())
                hb_sb.append(h)

            # resident part of A: [128, RESN, PC] fp16, ~80 KB/partition
            a_res = None
            if RESN:
                a_res = const.tile([128, RESN, PC], F16, name="a_res")
                for s4 in range(4):
                    sl = slice(s4 * (RESN // 4), (s4 + 1) * (RESN // 4))
                    nc.sync.dma_start(
                        a_res[:, sl, :],
                        ares_d.ap()[sl].rearrange("k p n -> p k n"))

            xt_full = persist.tile([128, KT, D], F16, name="xt_full",
                                   tag="xt_full")
            xtf_view = xt_full[:].rearrange("p (c t) d -> p c t d", t=NT)

            def sc(name):
                return persist.tile([128, NT], F32, name=name)

            def sumsq(src_ap, accum_ap):
                # SBUF src: square+reduce on DVE
                s = sqp.tile([128, D], F32, name="sq_scr", tag="sq_scr")
                nc.vector.tensor_tensor(s[:], src_ap, src_ap, ALU.mult)
                nc.vector.reduce_sum(accum_ap, s[:], axis=AX.X)

            def sumsq_gp(src_ap, accum_ap):
                # SBUF src: square on gpsimd, reduce on DVE (gpsimd cannot
                # reduce along the free axis)
                s = sqp.tile([128, D], F32, name="sg_scr", tag="sg_scr")
                nc.gpsimd.tensor_tensor(s[:], src_ap, src_ap, ALU.mult)
                nc.vector.reduce_sum(accum_ap, s[:], axis=AX.X)

            def sumsq_ps(src_ap, accum_ap):
                # PSUM src: Square+accum on the scalar engine
                s = sqp.tile([128, D], F32, name="sqp_scr", tag="sqp_scr")
                nc.scalar.activation(s[:], src_ap, AF.Square,
                                     accum_out=accum_ap)

            def sumsq_alt(src_ap, accum_ap, t):
                # alternate engines across tiles to halve the serial span
                if t % 2 == 0:
                    sumsq_ps(src_ap, accum_ap)
                else:
                    sumsq(src_ap, accum_ap)

            def s_sqrt(dst, src, name):
                # sqrt(x) = exp(0.5*ln(x)): Ln and Exp share one activation
                # table set, so chains never reload tables
                cl = sc(name + "_cl")
                nc.vector.tensor_scalar_max(cl[:], src[:], 1e-30)
                lg = sc(name + "_lg")
                nc.scalar.activation(lg[:], cl[:], AF.Ln)
                nc.scalar.activation(dst[:], lg[:], AF.Exp, scale=0.5)

            def s_tanh(dst, src, name, pre=1.0):
                # tanh(p*x) = 2/(1+exp(-2p*x)) - 1 for x >= 0
                ex = sc(name + "_ex")
                nc.scalar.activation(ex[:], src[:], AF.Exp, scale=-2.0 * pre)
                p1 = sc(name + "_p1")
                nc.vector.tensor_scalar_add(p1[:], ex[:], 1.0)
                r = sc(name + "_r")
                nc.vector.reciprocal(r[:], p1[:])
                nc.vector.tensor_scalar(dst[:], r[:], 2.0, -1.0,
                                        ALU.mult, ALU.add)

            def clamp_recip(dst, src, name):
                c = sc(name + "_c")
                nc.vector.tensor_scalar_max(c[:], src[:], MINN)
                nc.vector.reciprocal(dst[:], c[:])

            def artanh_ln(dst, x, name):
                """dst = ln((1+x)/(1-x)); caller owns the 0.5 factor."""
                ap1 = sc(name + "_ap")
                am1 = sc(name + "_am")
                ram = sc(name + "_ram")
                q = sc(name + "_q")
                nc.vector.tensor_scalar_add(ap1[:], x[:], 1.0)
                nc.vector.tensor_scalar(am1[:], x[:], -1.0, 1.0,
                                        ALU.mult, ALU.add)
                nc.vector.reciprocal(ram[:], am1[:])
                nc.vector.tensor_tensor(q[:], ap1[:], ram[:], ALU.mult)
                nc.scalar.activation(dst[:], q[:], AF.Ln)

            # -------- encode scalars: s0 = min(tanh|x|,MAXN)/|x| ----------
            # h = proj(expmap0(x)) = s0 * x is never materialized; s0 folds
            # into the layer's scalar chains.
            xn2 = sc("xn2")
            for t in range(NT):
                sumsq_alt(x_sb[:, t, :], xn2[:, t : t + 1], t)
            un = sc("un")
            s_sqrt(un, xn2, "enc_sq")
            run_ = sc("run")
            clamp_recip(run_, un, "enc_r")
            thx = sc("thx")
            s_tanh(thx, un, "enc_th")
            mn0 = sc("mn0")
            nc.vector.tensor_scalar_min(mn0[:], thx[:], MAXN)
            s0 = sc("s0")
            nc.vector.tensor_tensor(s0[:], mn0[:], run_[:], ALU.mult)

            def layer(li, v_in, s_in, hnorm, e_out_d, e_tag):
                """One HGCN layer on raw tangent input: the on-ball h is
                s_in * v_in with |h| = hnorm. Returns (xt2, ss, mm2) where
                the next layer's input is v=xt2 scaled by s=ss."""
                L = f"l{li}_"
                w = w_sb[li]
                hb = hb_sb[li]
                y2 = float(y2s[li])

                # ---- HypLinear matmuls on raw v: mx' = v @ [W^T | u] ----
                # the 257th output column is v.u (= mx.hb), so rypr rides
                # the same matmul; PD = 260 pads the PSUM row
                mxr_all = persist.tile([128, NT, D], F32, name=L + "mx",
                                       tag="bigB")
                mn2r = sc(L + "mn2r")
                rypr = sc(L + "rypr")
                for t in range(NT):
                    hT = htp.tile([128, 2, 128], F32, name="hT", tag="hT")
                    for kc in range(2):
                        psT = pst.tile([128, 128], F32, name="psT", tag="psT")
                        nc.tensor.transpose(
                            psT[:], v_in[:, t, kc * 128 : (kc + 1) * 128],
                            ident[:])
                        nc.vector.tensor_copy(hT[:, kc, :], psT[:])
                    pmx = psmx.tile([128, PD], F32, name="pmx", tag="pmx")
                    nc.tensor.matmul(pmx[:], hT[:, 0, :], w[:, 0, :],
                                     start=True, stop=False)
                    nc.tensor.matmul(pmx[:], hT[:, 1, :], w[:, 1, :],
                                     start=False, stop=True)
                    nc.vector.tensor_copy(mxr_all[:, t, :], pmx[:, 0:D])
                    nc.vector.tensor_copy(rypr[:, t : t + 1],
                                          pmx[:, D : D + 1])
                    if t % 2 == 0:
                        sumsq_ps(pmx[:, 0:D], mn2r[:, t : t + 1])
                    else:
                        sumsq(mxr_all[:, t, :], mn2r[:, t : t + 1])

                # ---- scale bookkeeping: mx = s_in*mxr ----
                s2 = sc(L + "s2")
                nc.vector.tensor_tensor(s2[:], s_in[:], s_in[:], ALU.mult)
                mn2 = sc(L + "mn2")
                nc.vector.tensor_tensor(mn2[:], mn2r[:], s2[:], ALU.mult)
                ryp = sc(L + "ryp")
                nc.vector.tensor_tensor(ryp[:], rypr[:], s_in[:], ALU.mult)

                # ---- SB1: mobius_matvec scalars ----
                mxn = sc(L + "mxn")
                s_sqrt(mxn, mn2, L + "mxn_sq")
                nc.vector.tensor_scalar_max(mxn[:], mxn[:], MINN)
                rxn = sc(L + "rxn")
                clamp_recip(rxn, hnorm, L + "rxn")
                rmxn = sc(L + "rmxn")
                nc.vector.reciprocal(rmxn[:], mxn[:])
                atx = sc(L + "atx")
                artanh_ln(atx, hnorm, L + "atx")
                targ = sc(L + "targ")
                nc.vector.tensor_tensor(targ[:], mxn[:], rxn[:], ALU.mult)
                nc.vector.tensor_tensor(targ[:], targ[:], atx[:], ALU.mult)
                th = sc(L + "th")
                s_tanh(th, targ, L + "th", pre=0.5)
                sres = sc(L + "sres")
                nc.vector.tensor_tensor(sres[:], th[:], rmxn[:], ALU.mult)
                # proj of res: norm is th (analytically); f1 = min(MAXN/th, 1)
                rth = sc(L + "rth")
                clamp_recip(rth, th, L + "rth")
                f1 = sc(L + "f1")
                nc.vector.tensor_scalar(f1[:], rth[:], MAXN, 1.0,
                                        ALU.mult, ALU.min)
                nres = sc(L + "nres")
                nc.vector.tensor_scalar_min(nres[:], th[:], MAXN)
                x2 = sc(L + "x2")
                nc.vector.tensor_tensor(x2[:], nres[:], nres[:], ALU.mult)

                # ---- SB2: mobius_add coefficients ----
                xy = sc(L + "xy")
                nc.vector.tensor_tensor(xy[:], ryp[:], sres[:], ALU.mult)
                nc.vector.tensor_tensor(xy[:], xy[:], f1[:], ALU.mult)
                apre = sc(L + "apre")
                nc.vector.tensor_scalar(apre[:], xy[:], 2.0, 1.0 + y2,
                                        ALU.mult, ALU.add)
                alpha = sc(L + "alpha")
                nc.vector.tensor_tensor(alpha[:], apre[:], f1[:], ALU.mult)
                beta = sc(L + "beta")
                nc.vector.tensor_scalar(beta[:], x2[:], -1.0, 1.0,
                                        ALU.mult, ALU.add)
                den = sc(L + "den")
                nc.vector.tensor_scalar(den[:], x2[:], y2, 1.0,
                                        ALU.mult, ALU.add)
                xy2 = sc(L + "xy2")
                nc.vector.tensor_scalar_mul(xy2[:], xy[:], 2.0)
                nc.vector.tensor_tensor(den[:], den[:], xy2[:], ALU.add)
                dinv = sc(L + "dinv")
                clamp_recip(dinv, den, L + "dinv")
                asc = sc(L + "asc")
                nc.vector.tensor_tensor(asc[:], alpha[:], dinv[:], ALU.mult)
                nc.vector.tensor_tensor(asc[:], asc[:], sres[:], ALU.mult)
                bsc = sc(L + "bsc")
                nc.vector.tensor_tensor(bsc[:], beta[:], dinv[:], ALU.mult)

                # ---- |h2|^2 analytically (h2 = asc*mx + bsc*hb) ----
                hn2 = sc(L + "hn2")
                a2 = sc(L + "a2")
                nc.vector.tensor_tensor(a2[:], asc[:], asc[:], ALU.mult)
                nc.vector.tensor_tensor(hn2[:], a2[:], mn2[:], ALU.mult)
                ab = sc(L + "ab")
                nc.vector.tensor_tensor(ab[:], asc[:], bsc[:], ALU.mult)
                abry = sc(L + "abry")
                nc.vector.tensor_tensor(abry[:], ab[:], ryp[:], ALU.mult)
                nc.vector.tensor_scalar_mul(abry[:], abry[:], 2.0)
                nc.vector.tensor_tensor(hn2[:], hn2[:], abry[:], ALU.add)
                b2 = sc(L + "b2")
                nc.vector.tensor_tensor(b2[:], bsc[:], bsc[:], ALU.mult)
                nc.vector.tensor_scalar_mul(b2[:], b2[:], y2)
                nc.vector.tensor_tensor(hn2[:], hn2[:], b2[:], ALU.add)

                # ---- SB3: proj + logmap0 scale ----
                hn = sc(L + "hn")
                s_sqrt(hn, hn2, L + "hn_sq")
                rhn = sc(L + "rhn")
                clamp_recip(rhn, hn, L + "rhn")
                f2 = sc(L + "f2")
                nc.vector.tensor_scalar(f2[:], rhn[:], MAXN, 1.0,
                                        ALU.mult, ALU.min)
                m = sc(L + "m")
                nc.vector.tensor_scalar_min(m[:], hn[:], MAXN)
                rm = sc(L + "rm")
                clamp_recip(rm, m, L + "rm")
                atm = sc(L + "atm")
                artanh_ln(atm, m, L + "atm")
                g0 = sc(L + "g0")
                nc.vector.tensor_tensor(g0[:], atm[:], rm[:], ALU.mult)
                g = sc(L + "g")
                nc.vector.tensor_tensor(g[:], g0[:], f2[:], ALU.mult)
                nc.vector.tensor_scalar_mul(g[:], g[:], 0.5)
                gas = sc(L + "gas")
                nc.vector.tensor_tensor(gas[:], g[:], asc[:], ALU.mult)
                nc.vector.tensor_tensor(gas[:], gas[:], s_in[:], ALU.mult)
                gb = sc(L + "gb")
                nc.vector.tensor_tensor(gb[:], g[:], bsc[:], ALU.mult)

                # ---- xt = gas*mxr + gb*hb (tangent features, fp16) ----
                xt_all = persist.tile([128, NT, D], F16, name=L + "xt",
                                      tag="bigC")
                for t in range(NT):
                    t2 = sqp.tile([128, D], F32, name="t2t", tag="t2t")
                    nc.gpsimd.tensor_scalar_mul(t2[:], hb[:],
                                                gb[:, t : t + 1])
                    t1 = sqp.tile([128, D], F32, name="t1t", tag="t1t")
                    nc.vector.tensor_scalar_mul(t1[:], mxr_all[:, t, :],
                                                gas[:, t : t + 1])
                    nc.vector.tensor_tensor(xt_all[:, t, :], t1[:], t2[:],
                                            ALU.add)

                # ---- AllGather tangent features (chunks of CH tiles) ----
                for j, sz in enumerate(CH):
                    off = CHOFF[j]
                    agin = dram.tile([sz, 128, D], F16, name=f"{L}agin{j}",
                                     tag=f"agin{j}")
                    agout = dram.tile([NCORES * sz, 128, D], F16,
                                      name=f"{L}agout{j}", tag=f"agout{j}",
                                      addr_space="Shared")
                    nc.scalar.dma_start(agin[:].rearrange("t p d -> p t d"),
                                        xt_all[:, off : off + sz, :])
                    nc.gpsimd.collective_compute(
                        "AllGather", ALU.bypass,
                        replica_groups=[list(range(NCORES))],
                        ins=[agin[:].opt()], outs=[agout[:].opt()])
                    agout_v = agout[:].rearrange("(c t) p d -> p c t d", t=sz)
                    for i in range(sz):
                        nc.scalar.dma_start(
                            xtf_view[:, :, off + i, :],
                            agout_v[:, :, i, :])

                # ---- spmm: agg[dst, f] = sum_src AT[src, dst] xt[src, f] ----
                # i-major so each AG back-DMA (per i, all cores) unblocks 8
                # k-tiles at a time; first RESN slots come from SBUF.
                # PSUM 'start' clears the whole 2KB bank; tiles t, t+1 share
                # a bank, so only the very first matmul of each even tile
                # issues start=True.
                pagg = psagg.tile([128, NT, D], F32, name="pagg", tag="pagg")
                k_iter = 0
                for j, sz in enumerate(CH):
                    for i in range(sz):
                        for c in range(NCORES):
                            kt = c * NT + CHOFF[j] + i
                            s = (CHOFF[j] + i) * NCORES + c
                            if s < RESN:
                                lhs = a_res[:, s, :]
                            else:
                                at_k = atp.tile([128, PC], F16, name="at_k",
                                                tag="at_k")
                                nc.sync.dma_start(at_k[:],
                                                  astr_d.ap()[s - RESN])
                                lhs = at_k[:]
                            for t in range(NT):
                                nc.tensor.matmul(
                                    pagg[:, t, :],
                                    lhs[:, t * 128 : (t + 1) * 128],
                                    xt_full[:, kt, :],
                                    start=(k_iter == 0 and t % 2 == 0),
                                    stop=(k_iter == KT - 1),
                                    skip_group_check=True)
                            k_iter += 1

                # ---- HypAct ----
                r2 = sc(L + "r2")
                for t in range(NT):
                    sumsq_ps(pagg[:, t, :], r2[:, t : t + 1])
                rn = sc(L + "rn")
                s_sqrt(rn, r2, L + "rn_sq")
                rrn = sc(L + "rrn")
                clamp_recip(rrn, rn, L + "rrn")
                th2 = sc(L + "th2")
                s_tanh(th2, rn, L + "th2")
                m1 = sc(L + "m1")
                nc.vector.tensor_scalar_min(m1[:], th2[:], MAXN)
                rm1 = sc(L + "rm1")
                clamp_recip(rm1, m1, L + "rm1")
                s1 = sc(L + "s1")
                nc.vector.tensor_tensor(s1[:], m1[:], rrn[:], ALU.mult)
                atq = sc(L + "atq")
                artanh_ln(atq, m1, L + "atq")
                qs0 = sc(L + "qs0")
                nc.vector.tensor_tensor(qs0[:], s1[:], atq[:], ALU.mult)
                qs = sc(L + "qs")
                nc.vector.tensor_tensor(qs[:], qs0[:], rm1[:], ALU.mult)
                nc.vector.tensor_scalar_mul(qs[:], qs[:], 0.5)

                xt2_all = persist.tile([128, NT, D], F32, name=L + "xt2",
                                       tag="bigD")
                n2b = sc(L + "n2b")
                for t in range(NT):
                    # relu(qs*agg) on DVE (PSUM src)
                    nc.vector.tensor_scalar(xt2_all[:, t, :], pagg[:, t, :],
                                            qs[:, t : t + 1], 0.0,
                                            ALU.mult, ALU.max)
                    sumsq_gp(xt2_all[:, t, :], n2b[:, t : t + 1])

                un2 = sc(L + "un2")
                s_sqrt(un2, n2b, L + "un2_sq")
                run2 = sc(L + "run2")
                clamp_recip(run2, un2, L + "run2")
                th3 = sc(L + "th3")
                s_tanh(th3, un2, L + "th3")
                mm2 = sc(L + "mm2")
                nc.vector.tensor_scalar_min(mm2[:], th3[:], MAXN)
                ss = sc(L + "ss")
                nc.vector.tensor_tensor(ss[:], mm2[:], run2[:], ALU.mult)

                # e = ss*xt2: off the critical path, only feeds the output
                e_all = persist.tile([128, NT, D], F32, name=L + "e",
                                     tag=e_tag)
                for t in range(NT):
                    nc.gpsimd.tensor_scalar_mul(e_all[:, t, :],
                                                xt2_all[:, t, :],
                                                ss[:, t : t + 1])
                ev = e_out_d.ap().rearrange("t p d -> p t d")
                for t0 in range(0, NT, 2):
                    nc.sync.dma_start(ev[:, t0 : t0 + 2, :],
                                      e_all[:, t0 : t0 + 2, :])
                return xt2_all, ss, mm2

            xt2_1, ss1, n1 = layer(0, x_sb, s0, mn0, e1_d, "bigA")
            layer(1, xt2_1, ss1, n1, e2_d, "bigA")

    nc.compile()
    return nc


def _hyp_bias(b):
    """proj(expmap0(b, c=1), c=1) in float32, mirroring the reference."""
    b = b.astype(np.float32)
    un = np.maximum(np.sqrt((b * b).sum()), np.float32(MINN)).astype(np.float32)
    h = (np.tanh(un) * b / un).astype(np.float32)
    n = np.maximum(np.sqrt((h * h).sum()), np.float32(MINN)).astype(np.float32)
    if n > np.float32(MAXN):
        h = (h / n * np.float32(MAXN)).astype(np.float32)
    return h


def prepare_inputs(x, W1, b1, W2, b2, edge_index, edge_weight):
    x = np.asarray(x, np.float32)
    W1 = np.asarray(W1, np.float32)
    W2 = np.asarray(W2, np.float32)
    b1 = np.asarray(b1, np.float32)
    b2 = np.asarray(b2, np.float32)
    ew = np.asarray(edge_weight, np.float32)
    src = np.asarray(edge_index[0], np.int64)
    dst = np.asarray(edge_index[1], np.int64)

    AT = np.zeros((NP, NP), np.float32)
    np.add.at(AT, (src, dst), ew)

    xfull = np.zeros((NP, D), np.float32)
    xfull[:N] = x

    hb1 = _hyp_bias(b1)
    hb2 = _hyp_bias(b2)
    y2s = (float((hb1.astype(np.float64) ** 2).sum()),
           float((hb2.astype(np.float64) ** 2).sum()))
    u1 = (W1.T.astype(np.float64) @ hb1.astype(np.float64)).astype(np.float32)
    u2 = (W2.T.astype(np.float64) @ hb2.astype(np.float64)).astype(np.float32)

    w1t = np.zeros((2, 128, 260), np.float32)
    w2t = np.zeros((2, 128, 260), np.float32)
    w1t[:, :, :D] = np.ascontiguousarray(W1.T).reshape(2, 128, D)
    w2t[:, :, :D] = np.ascontiguousarray(W2.T).reshape(2, 128, D)
    w1t[:, :, D] = u1.reshape(2, 128)
    w2t[:, :, D] = u2.reshape(2, 128)
    hb1_b = np.tile(hb1[None, :], (128, 1)).astype(np.float32)
    hb2_b = np.tile(hb2[None, :], (128, 1)).astype(np.float32)

    # consumption order: slot s = (j*TPC + i)*NCORES + c, kt = c*NT + j*TPC+i
    in_maps = []
    for core in range(NCORES):
        ac = np.ascontiguousarray(
            AT[:, core * PC : (core + 1) * PC]).reshape(KT, 128, PC)
        ac = ac.astype(np.float16)
        a_all = np.empty((KT, 128, PC), np.float16)
        for ti in range(NT):
            for c in range(NCORES):
                a_all[ti * NCORES + c] = ac[c * NT + ti]
        a_res = np.ascontiguousarray(a_all[:max(RESN, 1)])
        a_str = np.ascontiguousarray(a_all[RESN:])
        xcr = xfull[core * PC : (core + 1) * PC].reshape(NT, 128, D)
        in_maps.append({
            "xc": np.ascontiguousarray(xcr),
            "ares": a_res,
            "astr": a_str,
            "w1t": w1t, "w2t": w2t,
            "hb1": hb1_b, "hb2": hb2_b,
        })
    return in_maps, y2s


def assemble(results):
    e1 = np.concatenate([r["e1"].reshape(PC, D) for r in results], 0)[:N]
    e2 = np.concatenate([r["e2"].reshape(PC, D) for r in results], 0)[:N]
    return np.stack([e1, e2], 0).astype(np.float32)


def run(inputs, trace=False):
    in_maps, y2s = prepare_inputs(**inputs)
    nc = build_nc(y2s)
    res = run_bass_kernel_spmd(nc, in_maps, core_ids=list(range(NCORES)),
                               trace=trace)
    return assemble(res.results), res


def kernel(**inputs):
    out, _ = run(inputs, trace=False)
    return out
